# revision 13
# baseline (speedup 1.0000x reference)
"""Causal multi-head attention for Trainium2, head-sharded across 8 NeuronCores.

Reference computation (note the role swap: softmax rows are indexed by the
*key* position c and run over *query* positions C <= c):
    q = W_Q x ; k = W_K x ; v = W_V x            per head a
    S[c, C] = k[c] . q[C]
    attn = softmax_C( where(C <= c, S, -inf) / sqrt(64) )
    z[c]  = sum_C attn[c, C] v[C]
    out   = sum_a W_O[a] z[a]                     [seq, embed]

Sharding: 16 heads -> 2 heads per core.  Each core computes the partial
output for its two heads; the host sums the 8 partials.

Device-side design (per core; every matmul's moving operand is bf16 so the
PE streams 1 row/cycle at any width, and DMA traffic is halved):
    xT   [1024, 2048] bf16  x transposed (embed on partitions), streamed in
                            column-chunk-major pieces
    QT2  [128, 2048] bf16   both heads' q-projection, head-major on partitions
    KT2  [128, 2048] bf16   both heads' k-projection
    V1   [128, 2, 16, 65] bf16  v tiles [C, head, ct, h|ones] (col 64 = 1.0)

    V is projected directly in [C, h] orientation (lhsT = xT seq-tile,
    rhs = W_V) so no PE transposes are needed; the ones-column of V1 yields
    the softmax denominator as row 64 of the PV accumulator for free.

    Scores are computed transposed, ST[C, c] = lhsT(QT2).T @ rhs(KT2), so
    the PV contraction (over C) needs no transposes.  exp runs on ACT
    (the secondary bottleneck: ~0.83 ns/col + ~185 ns/instr), the causal
    mask multiply on DVE, PSUM->SBUF copies are spread over DVE (Q/K) and
    Pool (V, output), and all DMAs ride the SP HWDGE queue.

    The whole kernel is one software pipeline over the 4 512-wide c-chunks:
    each chunk's projections run as soon as its xT columns land, its
    attention (both heads' blocks interleaved, diagonal blocks shrunk to
    their unmasked columns) follows immediately, and its output projection
    is deferred into the next chunk's pipeline so no engine drains at the
    chunk boundary.  Output partials are written bf16 and summed on host.
"""

import numpy as np
import ml_dtypes

import concourse.bacc as bacc
import concourse.mybir as mybir
import concourse.tile as tile
from concourse import bass_utils

BATCH, SEQ, E, NH, H = 1, 2048, 1024, 16, 64
NCORES = 8
HPC = NH // NCORES          # heads per core
H2 = HPC * H                # 128, both heads' h packed
CS = 512                    # c-chunk (free-dim) width
NCS = SEQ // CS             # 4
NKT = E // 128              # 8 k-tiles over embed
NCT = SEQ // 128            # 16 C-tiles over sequence
SCALE = 1.0 / np.sqrt(H)    # 0.125
F32R = mybir.dt.float32r
F32 = mybir.dt.float32
BF16 = mybir.dt.bfloat16
NPBF16 = ml_dtypes.bfloat16

_built = None
CS_SET = None
NBLK_CAP = 99
DEPTH_OVERRIDE = 7


def _build(stage=5):
    nc = bacc.Bacc("TRN2", target_bir_lowering=False, debug=False)

    xT_d = nc.dram_tensor("xT", [E, SEQ], BF16, kind="ExternalInput").ap()
    wq_d = nc.dram_tensor("wq", [128, NKT, H2], BF16, kind="ExternalInput").ap()
    wk_d = nc.dram_tensor("wk", [128, NKT, H2], BF16, kind="ExternalInput").ap()
    wv_d = nc.dram_tensor("wv", [128, NKT, H2], BF16, kind="ExternalInput").ap()
    wo_d = nc.dram_tensor("wo", [H2, E], BF16, kind="ExternalInput").ap()
    masks_d = nc.dram_tensor("masks", [128, CS], BF16, kind="ExternalInput").ap()
    # ones pair (not single column): V1 slabs are padded to 66 bf16
    # elements = 132B so every slab and the ones-pair write are 4-byte
    # aligned -- a lone 2-byte scattered DMA write can RMW-clobber the
    # neighboring tile's first V element while V copies are in flight
    vcol_d = nc.dram_tensor("vcol", [128, HPC, NCT, 2], BF16,
                            kind="ExternalInput").ap()
    ones1_d = nc.dram_tensor("ones1", [1, H], F32R, kind="ExternalInput").ap()
    out_d = nc.dram_tensor("out", [SEQ, E], F32, kind="ExternalOutput").ap()

    def _body(tc):
        with (
            tc.tile_pool(name="persist", bufs=1) as persist,
            tc.tile_pool(name="work", bufs=3) as work,
            tc.tile_pool(name="zpool", bufs=4) as zpool,
            tc.tile_pool(name="opool", bufs=4) as opool,
            tc.tile_pool(name="ps_proj", bufs=2, space="PSUM") as ps_proj,
            tc.tile_pool(name="ps_s", bufs=2, space="PSUM") as ps_s,
            tc.tile_pool(name="ps_z", bufs=2, space="PSUM") as ps_z,
            tc.tile_pool(name="ps_o", bufs=2, space="PSUM") as ps_o,
        ):
            # ---- resident tensors -------------------------------------
            xT = persist.tile([128, NKT, SEQ], BF16)
            wq = persist.tile([128, NKT, H2], BF16)
            wk = persist.tile([128, NKT, H2], BF16)
            wv = persist.tile([128, NKT, H2], BF16)
            wo = persist.tile([128, E], BF16)
            ones1 = persist.tile([1, H], F32R)
            QT2 = persist.tile([128, SEQ], BF16)
            KT2 = persist.tile([128, SEQ], BF16)
            V1 = persist.tile([128, HPC, NCT, H + 2], BF16)
            mask_sb = persist.tile([128, CS], BF16)

            # small DMAs first so weights/constants never queue behind the
            # 4MB xT load; xT is issued column-chunk-major so chunk cc's
            # projections start as soon as its columns land
            nc.sync.dma_start(wq[:], wq_d[:])
            for k in range(NKT):
                nc.sync.dma_start(xT[:, k, 0:CS], xT_d[k * 128:(k + 1) * 128, 0:CS])
            nc.sync.dma_start(wk[:], wk_d[:])
            nc.sync.dma_start(wv[:], wv_d[:])
            nc.sync.dma_start(mask_sb[:], masks_d[:])
            nc.sync.dma_start(V1[:, :, :, H:H + 2], vcol_d[:])
            nc.sync.dma_start(ones1[:], ones1_d[:])
            nc.sync.dma_start(wo[:], wo_d[:])
            for cc in range(1, NCS):
                for k in range(NKT):
                    nc.sync.dma_start(
                        xT[:, k, cc * CS:(cc + 1) * CS],
                        xT_d[k * 128:(k + 1) * 128, cc * CS:(cc + 1) * CS])

            def emit_outproj_unit(cs, z2, mt):
                # out[c, e] = z2.T @ wo: K=128 sums both heads at once
                # (z2 rows 0:64 head0, 64:128 head1; wo rows match)
                for et in range(E // CS):
                    o_ps = ps_o.tile([128, CS], F32, name="o_ps")
                    nc.tensor.matmul(
                        o_ps[:],
                        z2[:, mt * 128:(mt + 1) * 128],
                        wo[:, et * CS:(et + 1) * CS],
                        start=True, stop=True,
                    )
                    o_sb = opool.tile([128, CS], F32, tag="o", name="o_sb")
                    nc.vector.tensor_copy(o_sb[:], o_ps[:])
                    nc.sync.dma_start(
                        out_d[cs * CS + mt * 128: cs * CS + (mt + 1) * 128,
                              et * CS:(et + 1) * CS],
                        o_sb[:],
                    )

            def outproj_units(cs, z2):
                return [(lambda mt=mt: emit_outproj_unit(cs, z2, mt))
                        for mt in range(CS // 128)]

            def emit_qk_proj(cc, w_sb, dstT, half):
                # one half-projection: accumulate 4 of the 8 embed k-tiles
                c0, c1 = cc * CS, (cc + 1) * CS
                if half == 0:
                    p_ps = ps_proj.tile([128, CS], F32, tag="proj",
                                        name="p_ps")
                    emit_qk_proj.live[(cc, id(dstT))] = p_ps
                else:
                    p_ps = emit_qk_proj.live.pop((cc, id(dstT)))
                for k in range(4 * half, 4 * half + 4):
                    nc.tensor.matmul(
                        p_ps[:], w_sb[:, k, :], xT[:, k, c0:c1],
                        start=(k == 0), stop=(k == NKT - 1),
                    )
                if half == 1:
                    nc.vector.tensor_copy(dstT[:, c0:c1], p_ps[:])
            emit_qk_proj.live = {}

            def emit_v_tile(cc, i):
                # V tile [C, h] directly via lhsT = xT seq-tile: no transpose
                ct = 4 * cc + i
                v_ps = ps_proj.tile([128, HPC, H], F32, tag="proj",
                                    name="v_ps")
                for k in range(NKT):
                    nc.tensor.matmul(
                        v_ps[:], xT[:, k, ct * 128:(ct + 1) * 128],
                        wv[:, k, :],
                        start=(k == 0), stop=(k == NKT - 1),
                    )
                nc.vector.tensor_copy(V1[:, :, ct, 0:H], v_ps[:])

            def proj_units(cc):
                return [
                    lambda: emit_qk_proj(cc, wq, QT2, 0),
                    lambda: emit_qk_proj(cc, wq, QT2, 1),
                    lambda: emit_qk_proj(cc, wk, KT2, 0),
                    lambda: emit_qk_proj(cc, wk, KT2, 1),
                    lambda: emit_v_tile(cc, 0),
                    lambda: emit_v_tile(cc, 1),
                    lambda: emit_v_tile(cc, 2),
                    lambda: emit_v_tile(cc, 3),
                ]

            filler = []
            cs_list = list(CS_SET if CS_SET is not None else range(NCS))
            for ci, cc in enumerate(cs_list):
                if ci == 0:
                    for f in proj_units(cc):
                        f()

                # ---- attention for cs=cc: both heads' blocks interleaved,
                # diagonal blocks shrunk to their unmasked columns ----------
                cs = cc
                nblk = min(4 * cs + 4, NBLK_CAP)
                z2 = zpool.tile([128, CS], BF16, tag="z", name="z2")
                z_ps = [ps_z.tile([H + 1, CS], F32, tag="zps",
                                  name=f"z_ps{hh}") for hh in range(HPC)]
                # stagger head1 two C-tiles ahead of head0 so the two
                # normalize chains at the end overlap instead of serializing
                OFF = min(2, nblk)
                blocks = []
                for t in range(nblk + OFF):
                    if t < nblk:
                        blocks.append((t, 1))
                    if t >= OFF:
                        blocks.append((t - OFF, 0))
                exp_tiles = {}
                DEPTH = DEPTH_OVERRIDE

                def do_score(i):
                    ct, hh = blocks[i]
                    h0 = hh * H
                    d = ct - 4 * cs
                    off = 128 * d if d > 0 else 0
                    n = CS - off
                    s_ps = ps_s.tile([128, CS], F32, tag="s", name="s_ps")
                    nc.tensor.matmul(
                        s_ps[:, 0:n],
                        QT2[h0:h0 + H, ct * 128:(ct + 1) * 128],
                        KT2[h0:h0 + H, cs * CS + off:(cs + 1) * CS],
                        start=True, stop=True,
                    )
                    e_sb = work.tile([128, CS], BF16, tag="exp",
                                     bufs=8, name="e_sb")
                    nc.scalar.activation(
                        e_sb[:, 0:n], s_ps[:, 0:n],
                        mybir.ActivationFunctionType.Exp, scale=SCALE,
                    )
                    if d >= 0:
                        # causal: within the shrunk block keep where p <= j.
                        # Pool (gpsimd) is otherwise idle and can handle the
                        # SBUF-only multiplies; cs=0's masks stay on DVE since
                        # every cs=0 block is masked and Pool can't keep pace
                        eng = nc.vector if cs == 0 else nc.gpsimd
                        eng.tensor_tensor(
                            e_sb[:, 0:n], e_sb[:, 0:n], mask_sb[:, 0:n],
                            op=mybir.AluOpType.mult,
                        )
                    exp_tiles[i] = (e_sb, off, n)

                def do_normalize(hh):
                    # z[h, c] /= z[64, c], via reciprocal + ones-broadcast
                    recip = work.tile([1, CS], F32R, tag="recip",
                                      name="recip")
                    with nc.allow_low_precision("float32r ~ fp32"):
                        nc.vector.reciprocal(recip[:], z_ps[hh][H:H + 1, :])
                    b_ps = ps_s.tile([H, CS], F32, tag="s", name="b_ps")
                    nc.tensor.matmul(b_ps[:], ones1[:], recip[:],
                                     start=True, stop=True)
                    bc_sb = work.tile([H, CS], F32R, tag="bc", name="bc_sb")
                    nc.scalar.activation(bc_sb[:], b_ps[:],
                                         mybir.ActivationFunctionType.Copy)
                    nc.vector.tensor_tensor(
                        z2[hh * H:(hh + 1) * H, :], z_ps[hh][0:H, :],
                        bc_sb[:], op=mybir.AluOpType.mult,
                    )

                def do_pv(i):
                    ct, hh = blocks[i]
                    e_sb, off, n = exp_tiles.pop(i)
                    nc.tensor.matmul(
                        z_ps[hh][:, off:CS], V1[:, hh, ct, 0:H + 1],
                        e_sb[:, 0:n],
                        start=(ct == 0), stop=(ct == nblk - 1),
                    )
                    if ct == nblk - 1:
                        do_normalize(hh)

                # drain filler (prev chunk's outproj + NEXT chunk's
                # projections) into this chunk's score/PV pipeline so PE
                # has independent work while ACT computes the exps
                if ci + 1 < len(cs_list):
                    filler.extend(proj_units(cs_list[ci + 1]))
                nb = len(blocks)
                last_cs = ci == len(cs_list) - 1
                # on the last chunk, hold filler back for the PV tail where
                # no score work is left to hide the exp latency
                hold = min(len(filler), DEPTH) if last_cs else 0
                for i in range(nb):
                    do_score(i)
                    if len(filler) > hold and (i % 2 == 1
                                               or nb - i <= len(filler) - hold):
                        filler.pop(0)()
                    if i >= DEPTH:
                        do_pv(i - DEPTH)
                for i in range(max(0, nb - DEPTH), nb):
                    do_pv(i)
                    if filler:
                        filler.pop(0)()
                while filler:
                    filler.pop(0)()

                if stage <= 4:
                    dbg = opool.tile([128, CS], F32, tag="o", name="dbg")
                    nc.vector.tensor_copy(dbg[:], z2[:])
                    nc.sync.dma_start(
                        out_d[cs * 128:(cs + 1) * 128, 0:CS], dbg[:])
                elif stage >= 5:
                    filler.extend(outproj_units(cs, z2))
            while filler:
                filler.pop(0)()

    with tile.TileContext(nc) as tc:
        _body(tc)
    nc.finalize()
    return nc


def _prep_inputs(x, W_Q, W_K, W_V, W_O):
    x = np.asarray(x, dtype=np.float32)
    W_Q = np.asarray(W_Q, dtype=np.float32)
    W_K = np.asarray(W_K, dtype=np.float32)
    W_V = np.asarray(W_V, dtype=np.float32)
    W_O = np.asarray(W_O, dtype=np.float32)

    xT = np.ascontiguousarray(x[0].T).astype(NPBF16)       # [E, SEQ]

    def swz(w):
        # [E, H2] -> [128(p), NKT(k), H2]: p-major so DMA rows are 2KB
        return np.ascontiguousarray(
            w.reshape(NKT, 128, H2).transpose(1, 0, 2)).astype(NPBF16)

    in_maps = []
    for c in range(NCORES):
        a0, a1 = HPC * c, HPC * c + 1
        # [E, 2h]: head0's 64 cols then head1's
        wq = swz(np.concatenate([W_Q[a0].T, W_Q[a1].T], axis=1))
        wk = swz(np.concatenate([W_K[a0].T, W_K[a1].T], axis=1))
        wv = swz(np.concatenate([W_V[a0].T, W_V[a1].T], axis=1))
        # [2h, E]
        wo = np.ascontiguousarray(
            np.concatenate([W_O[a0].T, W_O[a1].T], axis=0)).astype(NPBF16)
        in_maps.append({"xT": xT, "wq": wq, "wk": wk, "wv": wv, "wo": wo,
                        "masks": _MASKS, "vcol": _VCOL, "ones1": _ONES1})
    return in_maps


_MASKS = (np.arange(128)[:, None] <= np.arange(CS)[None, :]).astype(NPBF16)
_VCOL = np.ones((128, HPC, NCT, 2), dtype=NPBF16)
_ONES1 = np.ones((1, H), dtype=np.float32)


def _run(in_maps, trace=False):
    global _built
    if _built is None:
        _built = _build()
    res = bass_utils.run_bass_kernel_spmd(
        _built, in_maps, core_ids=list(range(NCORES)), trace=trace,
    )
    return res


def kernel(x, W_Q, W_K, W_V, W_O):
    in_maps = _prep_inputs(x, W_Q, W_K, W_V, W_O)
    res = _run(in_maps, trace=False)
    acc = np.zeros((SEQ, E), dtype=np.float64)
    for c in range(NCORES):
        acc += np.asarray(res.results[c]["out"], dtype=np.float64)
    return acc.astype(np.float32)[None, :, :]


def kernel_traced(x, W_Q, W_K, W_V, W_O):
    """Like kernel() but also returns a per-core exec-time estimate in ns.

    Prefers a real NTFF profile when the runtime supports it; otherwise
    falls back to the cost-model device-occupancy timeline (TimelineSim),
    since the axon client in this container has no NTFF hook.
    """
    in_maps = _prep_inputs(x, W_Q, W_K, W_V, W_O)
    exec_ns = None
    try:
        res = _run(in_maps, trace=True)
        exec_ns = res.exec_time_ns
    except Exception:
        res = _run(in_maps, trace=False)
    if exec_ns is None:
        from concourse.timeline_sim import TimelineSim
        exec_ns = int(TimelineSim(_built, trace=False).simulate())
    acc = np.zeros((SEQ, E), dtype=np.float64)
    for c in range(NCORES):
        acc += np.asarray(res.results[c]["out"], dtype=np.float64)
    return acc.astype(np.float32)[None, :, :], exec_ns


# revision 28
# speedup vs baseline: 1.0679x; 1.0679x over previous
"""Causal multi-head attention for Trainium2, head-sharded across 8 NeuronCores.

Reference computation (note the role swap: softmax rows are indexed by the
*key* position c and run over *query* positions C <= c):
    q = W_Q x ; k = W_K x ; v = W_V x            per head a
    S[c, C] = k[c] . q[C]
    attn = softmax_C( where(C <= c, S, -inf) / sqrt(64) )
    z[c]  = sum_C attn[c, C] v[C]
    out   = sum_a W_O[a] z[a]                     [seq, embed]

Sharding: 16 heads -> 2 heads per core.  Each core computes the partial
output for its two heads; the host sums the 8 partials.

Device-side design (per core; every matmul's moving operand is bf16 so the
PE streams 1 row/cycle at any width, and DMA traffic is halved):
    xT   [1024, 2048] bf16  x transposed (embed on partitions), streamed in
                            column-chunk-major pieces
    QT2  [128, 2048] bf16   both heads' q-projection, head-major on partitions
    KT2  [128, 2048] bf16   both heads' k-projection
    V1   [128, 2, 16, 65] bf16  v tiles [C, head, ct, h|ones] (col 64 = 1.0)

    V is projected directly in [C, h] orientation (lhsT = xT seq-tile,
    rhs = W_V) so no PE transposes are needed; the ones-column of V1 yields
    the softmax denominator as row 64 of the PV accumulator for free.

    Scores are computed transposed, ST[C, c] = lhsT(QT2).T @ rhs(KT2), so
    the PV contraction (over C) needs no transposes.  exp runs on ACT
    (the secondary bottleneck: ~0.83 ns/col + ~185 ns/instr), the causal
    mask multiply on DVE, PSUM->SBUF copies are spread over DVE (Q/K) and
    Pool (V, output), and all DMAs ride the SP HWDGE queue.

    The whole kernel is one software pipeline over the 4 512-wide c-chunks:
    each chunk's projections run as soon as its xT columns land, its
    attention (both heads' blocks interleaved, diagonal blocks shrunk to
    their unmasked columns) follows immediately, and its output projection
    is deferred into the next chunk's pipeline so no engine drains at the
    chunk boundary.  Output partials are written bf16 and summed on host.
"""

import numpy as np
import ml_dtypes

import concourse.bacc as bacc
import concourse.mybir as mybir
import concourse.tile as tile
from concourse import bass_utils

BATCH, SEQ, E, NH, H = 1, 2048, 1024, 16, 64
NCORES = 8
HPC = NH // NCORES          # heads per core
H2 = HPC * H                # 128, both heads' h packed
CS = 512                    # c-chunk (free-dim) width
NCS = SEQ // CS             # 4
NKT = E // 128              # 8 k-tiles over embed
NCT = SEQ // 128            # 16 C-tiles over sequence
SCALE = 1.0 / np.sqrt(H)    # 0.125
F32R = mybir.dt.float32r
F32 = mybir.dt.float32
BF16 = mybir.dt.bfloat16
NPBF16 = ml_dtypes.bfloat16

_built = None
CS_SET = None
NBLK_CAP = 99
DEPTH_OVERRIDE = 9
HOLD_TAIL = 2           # filler units held back for every chunk's normalize
PS_S_BUFS = 2
PS_O_BUFS = 2
HS = CS // 2            # normalize/outproj column-split width (256)


def _build(stage=5):
    nc = bacc.Bacc("TRN2", target_bir_lowering=False, debug=False)

    xT_d = nc.dram_tensor("xT", [E, SEQ], BF16, kind="ExternalInput").ap()
    wq_d = nc.dram_tensor("wq", [128, NKT, H2], BF16, kind="ExternalInput").ap()
    wk_d = nc.dram_tensor("wk", [128, NKT, H2], BF16, kind="ExternalInput").ap()
    wv_d = nc.dram_tensor("wv", [128, NKT, H2], BF16, kind="ExternalInput").ap()
    wo_d = nc.dram_tensor("wo", [H2, E], BF16, kind="ExternalInput").ap()
    masks_d = nc.dram_tensor("masks", [128, CS], BF16, kind="ExternalInput").ap()
    # ones pair (not single column): V1 slabs are padded to 66 bf16
    # elements = 132B so every slab and the ones-pair write are 4-byte
    # aligned -- a lone 2-byte scattered DMA write can RMW-clobber the
    # neighboring tile's first V element while V copies are in flight
    vcol_d = nc.dram_tensor("vcol", [128, HPC, NCT, 2], BF16,
                            kind="ExternalInput").ap()
    ones1_d = nc.dram_tensor("ones1", [1, H], F32R, kind="ExternalInput").ap()
    out_d = nc.dram_tensor("out", [SEQ, E], BF16, kind="ExternalOutput").ap()

    def _body(tc):
        with (
            tc.tile_pool(name="persist", bufs=1) as persist,
            tc.tile_pool(name="work", bufs=3) as work,
            tc.tile_pool(name="zpool", bufs=4) as zpool,
            tc.tile_pool(name="opool", bufs=4) as opool,
            tc.tile_pool(name="ps_proj", bufs=2, space="PSUM") as ps_proj,
            tc.tile_pool(name="ps_s", bufs=PS_S_BUFS, space="PSUM") as ps_s,
            tc.tile_pool(name="ps_z", bufs=2, space="PSUM") as ps_z,
            tc.tile_pool(name="ps_o", bufs=PS_O_BUFS, space="PSUM") as ps_o,
        ):
            # ---- resident tensors -------------------------------------
            xT = persist.tile([128, NKT, SEQ], BF16)
            wq = persist.tile([128, NKT, H2], BF16)
            wk = persist.tile([128, NKT, H2], BF16)
            wv = persist.tile([128, NKT, H2], BF16)
            wo = persist.tile([128, E], BF16)
            ones1 = persist.tile([1, H], F32R)
            QT2 = persist.tile([128, SEQ], BF16)
            KT2 = persist.tile([128, SEQ], BF16)
            V1 = persist.tile([128, HPC, NCT, H + 2], BF16)
            mask_sb = persist.tile([128, CS], BF16)

            # small DMAs first so weights/constants never queue behind the
            # 4MB xT load; xT is issued column-chunk-major so chunk cc's
            # projections start as soon as its columns land.  Each DMA costs
            # 625ns of serialized HWDGE setup, so chunks 1-3 load as a single
            # 3D-AP DMA each; chunk 0 stays per-k so the first projection
            # matmul can start after one small piece
            # DMA issue order tracks first-use time: HWDGE setup is 625ns of
            # serial issue per DMA, so both the issue horizon and the count
            # matter.  Chunk 0 lands in 2-k-tile pieces so the first
            # projection matmul starts early; chunk 1 in two 4-k-tile pieces
            # (its projections are chunk 0's attention filler); chunks 2-3
            # as one DMA each.
            xTv = xT_d.rearrange("(k p) c -> p k c", p=128)
            nc.sync.dma_start(wq[:, 0:NKT // 2, :], wq_d[:, 0:NKT // 2, :])
            for k2 in range(NKT // 2):
                nc.sync.dma_start(xT[:, 2 * k2:2 * k2 + 2, 0:CS],
                                  xTv[:, 2 * k2:2 * k2 + 2, 0:CS])
            nc.sync.dma_start(wq[:, NKT // 2:NKT, :], wq_d[:, NKT // 2:NKT, :])
            nc.sync.dma_start(wk[:], wk_d[:])
            nc.sync.dma_start(xT[:, 0:NKT // 2, CS:2 * CS],
                              xTv[:, 0:NKT // 2, CS:2 * CS])
            nc.sync.dma_start(wv[:], wv_d[:])
            nc.sync.dma_start(mask_sb[:], masks_d[:])
            nc.sync.dma_start(xT[:, NKT // 2:NKT, CS:2 * CS],
                              xTv[:, NKT // 2:NKT, CS:2 * CS])
            nc.sync.dma_start(V1[:, :, :, H:H + 2], vcol_d[:])
            nc.sync.dma_start(ones1[:], ones1_d[:])
            nc.sync.dma_start(wo[:], wo_d[:])
            for cc in range(2, NCS):
                nc.sync.dma_start(xT[:, :, cc * CS:(cc + 1) * CS],
                                  xTv[:, :, cc * CS:(cc + 1) * CS])

            def emit_outproj_unit(cs, z2, mt):
                # out[c, e] = z2.T @ wo: K=128 sums both heads at once
                # (z2 rows 0:64 head0, 64:128 head1; wo rows match).
                # Both 512-wide E halves share one 2-bank PSUM tile, one
                # PSUM->SBUF copy and one DMA (HWDGE setup is 625ns each)
                o_sb = opool.tile([128, E], BF16, tag="o", name="o_sb")
                for et in range(E // CS):
                    o_ps = ps_o.tile([128, CS], F32, name="o_ps")
                    nc.tensor.matmul(
                        o_ps[:],
                        z2[:, mt * 128:(mt + 1) * 128],
                        wo[:, et * CS:(et + 1) * CS],
                        start=True, stop=True,
                    )
                    nc.vector.tensor_copy(o_sb[:, et * CS:(et + 1) * CS],
                                          o_ps[:])
                nc.sync.dma_start(
                    out_d[cs * CS + mt * 128: cs * CS + (mt + 1) * 128, :],
                    o_sb[:],
                )

            def outproj_units(cs, z2, mts):
                return [(("out", cs), (lambda mt=mt: emit_outproj_unit(cs, z2, mt)))
                        for mt in mts]

            def emit_qk_proj(cc, w_sb, dstT, half):
                # one half-projection: accumulate 4 of the 8 embed k-tiles
                c0, c1 = cc * CS, (cc + 1) * CS
                if half == 0:
                    p_ps = ps_proj.tile([128, CS], F32, tag="proj",
                                        name="p_ps")
                    emit_qk_proj.live[(cc, id(dstT))] = p_ps
                else:
                    p_ps = emit_qk_proj.live.pop((cc, id(dstT)))
                for k in range(4 * half, 4 * half + 4):
                    nc.tensor.matmul(
                        p_ps[:], w_sb[:, k, :], xT[:, k, c0:c1],
                        start=(k == 0), stop=(k == NKT - 1),
                    )
                if half == 1:
                    nc.vector.tensor_copy(dstT[:, c0:c1], p_ps[:])
            emit_qk_proj.live = {}

            def emit_v_tile(cc, i):
                # V tile [C, h] directly via lhsT = xT seq-tile: no transpose
                ct = 4 * cc + i
                v_ps = ps_proj.tile([128, HPC, H], F32, tag="proj",
                                    name="v_ps")
                for k in range(NKT):
                    nc.tensor.matmul(
                        v_ps[:], xT[:, k, ct * 128:(ct + 1) * 128],
                        wv[:, k, :],
                        start=(k == 0), stop=(k == NKT - 1),
                    )
                nc.vector.tensor_copy(V1[:, :, ct, 0:H], v_ps[:])

            def proj_units(cc):
                units = [
                    lambda: emit_qk_proj(cc, wq, QT2, 0),
                    lambda: emit_qk_proj(cc, wq, QT2, 1),
                    lambda: emit_qk_proj(cc, wk, KT2, 0),
                    lambda: emit_qk_proj(cc, wk, KT2, 1),
                    lambda: emit_v_tile(cc, 0),
                    lambda: emit_v_tile(cc, 1),
                    lambda: emit_v_tile(cc, 2),
                    lambda: emit_v_tile(cc, 3),
                ]
                return [(("proj", cc), u) for u in units]

            filler = []
            cs_list = list(CS_SET if CS_SET is not None else range(NCS))
            for ci, cc in enumerate(cs_list):
                if ci == 0:
                    for _, f in proj_units(cc):
                        f()

                # ---- attention for cs=cc: both heads' blocks interleaved,
                # diagonal blocks shrunk to their unmasked columns ----------
                cs = cc
                nblk = min(4 * cs + 4, NBLK_CAP)
                z2 = zpool.tile([128, CS], BF16, tag="z", name="z2")
                z_ps = [ps_z.tile([H + 1, CS], F32, tag="zps",
                                  name=f"z_ps{hh}") for hh in range(HPC)]
                # stagger head1 two C-tiles ahead of head0 so the two
                # normalize chains at the end overlap instead of serializing
                OFF = min(2, nblk)
                blocks = []
                for t in range(nblk + OFF):
                    if t < nblk:
                        blocks.append((t, 1))
                    if t >= OFF:
                        blocks.append((t - OFF, 0))
                exp_tiles = {}
                DEPTH = DEPTH_OVERRIDE

                def do_score(i):
                    ct, hh = blocks[i]
                    h0 = hh * H
                    d = ct - 4 * cs
                    off = 128 * d if d > 0 else 0
                    n = CS - off
                    s_ps = ps_s.tile([128, CS], F32, tag="s", name="s_ps")
                    nc.tensor.matmul(
                        s_ps[:, 0:n],
                        QT2[h0:h0 + H, ct * 128:(ct + 1) * 128],
                        KT2[h0:h0 + H, cs * CS + off:(cs + 1) * CS],
                        start=True, stop=True,
                    )
                    e_sb = work.tile([128, CS], BF16, tag="exp",
                                     bufs=12, name="e_sb")
                    nc.scalar.activation(
                        e_sb[:, 0:n], s_ps[:, 0:n],
                        mybir.ActivationFunctionType.Exp, scale=SCALE,
                    )
                    if d >= 0:
                        # causal: within the shrunk block keep where p <= j.
                        # The diagonal blocks cluster at each chunk's end, so
                        # alternate the SBUF-only multiplies between DVE and
                        # the otherwise-idle Pool to halve the burst rate
                        eng = nc.vector if (d + hh) % 2 == 0 else nc.gpsimd
                        eng.tensor_tensor(
                            e_sb[:, 0:n], e_sb[:, 0:n], mask_sb[:, 0:n],
                            op=mybir.AluOpType.mult,
                        )
                    exp_tiles[i] = (e_sb, off, n)

                # normalize z[h, c] /= z[64, c] per 256-wide column half: the
                # left half's PV sum is complete once the d=1 diagonal block
                # lands, so its normalize + output projection overlap the
                # rest of the chunk instead of serializing at the boundary.
                # Only the reciprocal is emitted at the trigger PV; the PE
                # broadcast + copy + multiply become a deferred filler unit
                # so the in-order PE queue never parks on the reciprocal.
                norm_ran = [0, 0]

                def make_norm(hh, side, recip):
                    c0, c1 = side * HS, side * HS + HS

                    def run():
                        b_ps = ps_s.tile([H, HS], F32, tag="s", name="b_ps")
                        nc.tensor.matmul(b_ps[:], ones1[:], recip[:],
                                         start=True, stop=True)
                        bc_sb = work.tile([H, HS], F32R, tag="bc",
                                          name="bc_sb")
                        nc.scalar.activation(bc_sb[:], b_ps[:],
                                             mybir.ActivationFunctionType.Copy)
                        nc.vector.tensor_tensor(
                            z2[hh * H:(hh + 1) * H, c0:c1],
                            z_ps[hh][0:H, c0:c1],
                            bc_sb[:], op=mybir.AluOpType.mult,
                        )
                        norm_ran[hh] |= 1 << side
                        if norm_ran[0] & norm_ran[1] & (1 << side):
                            filler.extend(outproj_units(
                                cs, z2, (0, 1) if side == 0 else (2, 3)))
                    return run

                def queue_norm(hh, side):
                    c0, c1 = side * HS, side * HS + HS
                    recip = work.tile([1, HS], F32R, tag="recip",
                                      name="recip")
                    with nc.allow_low_precision("float32r ~ fp32"):
                        nc.vector.reciprocal(recip[:], z_ps[hh][H:H + 1, c0:c1])
                    filler.insert(min(1, len(filler)),
                                  (("norm", cs), make_norm(hh, side, recip)))

                def do_pv(i):
                    ct, hh = blocks[i]
                    e_sb, off, n = exp_tiles.pop(i)
                    nc.tensor.matmul(
                        z_ps[hh][:, off:CS], V1[:, hh, ct, 0:H + 1],
                        e_sb[:, 0:n],
                        start=(ct == 0), stop=(ct == nblk - 1),
                        skip_group_check=True,
                    )
                    if ct == min(4 * cs + 1, nblk - 1):
                        queue_norm(hh, 0)
                    if ct == nblk - 1:
                        queue_norm(hh, 1)

                # drain filler (deferred outproj + NEXT chunk's projections)
                # into this chunk's score/PV pipeline so PE has independent
                # work while ACT computes the exps
                if ci + 1 < len(cs_list):
                    filler.extend(proj_units(cs_list[ci + 1]))
                nb = len(blocks)
                last_cs = ci == len(cs_list) - 1
                # keep a few units in reserve for the chunk's normalize tail,
                # where no score work is left to hide the exp/recip latency
                hold = min(len(filler), DEPTH if last_cs else HOLD_TAIL)
                for i in range(nb):
                    do_score(i)
                    if len(filler) > hold and (i % 2 == 1
                                               or nb - i <= len(filler) - hold):
                        filler.pop(0)[1]()
                    if i >= DEPTH:
                        do_pv(i - DEPTH)
                for i in range(max(0, nb - DEPTH), nb):
                    do_pv(i)
                    if len(filler) > 1 or (filler and last_cs):
                        filler.pop(0)[1]()
                # leftover filler (this chunk's right-half outproj) carries
                # into the next chunk's pipeline -- except projections the
                # next chunk's scores depend on, and this chunk's normalize
                # units (the next chunk's first PV recycles the z_ps pool
                # buffers, which would deadlock the in-order PE queue)
                nxt = cs_list[ci + 1] if ci + 1 < len(cs_list) else None
                if nxt is not None:
                    while any(tag in (("proj", nxt), ("norm", cs))
                              for tag, _ in filler):
                        filler.pop(0)[1]()
            while filler:
                filler.pop(0)[1]()

    with tile.TileContext(nc) as tc:
        _body(tc)
    nc.finalize()
    return nc


def _prep_inputs(x, W_Q, W_K, W_V, W_O):
    x = np.asarray(x, dtype=np.float32)
    W_Q = np.asarray(W_Q, dtype=np.float32)
    W_K = np.asarray(W_K, dtype=np.float32)
    W_V = np.asarray(W_V, dtype=np.float32)
    W_O = np.asarray(W_O, dtype=np.float32)

    xT = np.ascontiguousarray(x[0].T).astype(NPBF16)       # [E, SEQ]

    def swz(w):
        # [E, H2] -> [128(p), NKT(k), H2]: p-major so DMA rows are 2KB
        return np.ascontiguousarray(
            w.reshape(NKT, 128, H2).transpose(1, 0, 2)).astype(NPBF16)

    in_maps = []
    for c in range(NCORES):
        a0, a1 = HPC * c, HPC * c + 1
        # [E, 2h]: head0's 64 cols then head1's
        wq = swz(np.concatenate([W_Q[a0].T, W_Q[a1].T], axis=1))
        wk = swz(np.concatenate([W_K[a0].T, W_K[a1].T], axis=1))
        wv = swz(np.concatenate([W_V[a0].T, W_V[a1].T], axis=1))
        # [2h, E]
        wo = np.ascontiguousarray(
            np.concatenate([W_O[a0].T, W_O[a1].T], axis=0)).astype(NPBF16)
        in_maps.append({"xT": xT, "wq": wq, "wk": wk, "wv": wv, "wo": wo,
                        "masks": _MASKS, "vcol": _VCOL, "ones1": _ONES1})
    return in_maps


_MASKS = (np.arange(128)[:, None] <= np.arange(CS)[None, :]).astype(NPBF16)
_VCOL = np.ones((128, HPC, NCT, 2), dtype=NPBF16)
_ONES1 = np.ones((1, H), dtype=np.float32)


def _run(in_maps, trace=False):
    global _built
    if _built is None:
        _built = _build()
    res = bass_utils.run_bass_kernel_spmd(
        _built, in_maps, core_ids=list(range(NCORES)), trace=trace,
    )
    return res


def kernel(x, W_Q, W_K, W_V, W_O):
    in_maps = _prep_inputs(x, W_Q, W_K, W_V, W_O)
    res = _run(in_maps, trace=False)
    acc = np.zeros((SEQ, E), dtype=np.float64)
    for c in range(NCORES):
        acc += np.asarray(res.results[c]["out"], dtype=np.float64)
    return acc.astype(np.float32)[None, :, :]


def kernel_traced(x, W_Q, W_K, W_V, W_O):
    """Like kernel() but also returns a per-core exec-time estimate in ns.

    Prefers a real NTFF profile when the runtime supports it; otherwise
    falls back to the cost-model device-occupancy timeline (TimelineSim),
    since the axon client in this container has no NTFF hook.
    """
    in_maps = _prep_inputs(x, W_Q, W_K, W_V, W_O)
    exec_ns = None
    try:
        res = _run(in_maps, trace=True)
        exec_ns = res.exec_time_ns
    except Exception:
        res = _run(in_maps, trace=False)
    if exec_ns is None:
        from concourse.timeline_sim import TimelineSim
        exec_ns = int(TimelineSim(_built, trace=False).simulate())
    acc = np.zeros((SEQ, E), dtype=np.float64)
    for c in range(NCORES):
        acc += np.asarray(res.results[c]["out"], dtype=np.float64)
    return acc.astype(np.float32)[None, :, :], exec_ns


# revision 39
# speedup vs baseline: 1.0868x; 1.0177x over previous
"""Causal multi-head attention for Trainium2, head-sharded across 8 NeuronCores.

Reference computation (note the role swap: softmax rows are indexed by the
*key* position c and run over *query* positions C <= c):
    q = W_Q x ; k = W_K x ; v = W_V x            per head a
    S[c, C] = k[c] . q[C]
    attn = softmax_C( where(C <= c, S, -inf) / sqrt(64) )
    z[c]  = sum_C attn[c, C] v[C]
    out   = sum_a W_O[a] z[a]                     [seq, embed]

Sharding: 16 heads -> 2 heads per core.  Each core computes the partial
output for its two heads; the host sums the 8 partials.

Device-side design (per core; every matmul's moving operand is bf16 so the
PE streams 1 row/cycle at any width, and DMA traffic is halved):
    xT   [1024, 2048] bf16  x transposed (embed on partitions), streamed in
                            column-chunk-major pieces
    QT2  [128, 2048] bf16   both heads' q-projection, head-major on partitions
    KT2  [128, 2048] bf16   both heads' k-projection
    V1   [128, 2, 16, 65] bf16  v tiles [C, head, ct, h|ones] (col 64 = 1.0)

    V is projected directly in [C, h] orientation (lhsT = xT seq-tile,
    rhs = W_V) so no PE transposes are needed; the ones-column of V1 yields
    the softmax denominator as row 64 of the PV accumulator for free.

    Scores are computed transposed, ST[C, c] = lhsT(QT2).T @ rhs(KT2), so
    the PV contraction (over C) needs no transposes.  exp runs on ACT
    (the secondary bottleneck: ~0.83 ns/col + ~185 ns/instr), the causal
    mask multiply on DVE, PSUM->SBUF copies are spread over DVE (Q/K) and
    Pool (V, output), and all DMAs ride the SP HWDGE queue.

    The whole kernel is one software pipeline over the 4 512-wide c-chunks:
    each chunk's projections run as soon as its xT columns land, its
    attention (both heads' blocks interleaved, diagonal blocks shrunk to
    their unmasked columns) follows immediately, and its output projection
    is deferred into the next chunk's pipeline so no engine drains at the
    chunk boundary.  Output partials are written bf16 and summed on host.
"""

import numpy as np
import ml_dtypes

import concourse.bacc as bacc
import concourse.mybir as mybir
import concourse.tile as tile
from concourse import bass_utils

BATCH, SEQ, E, NH, H = 1, 2048, 1024, 16, 64
NCORES = 8
HPC = NH // NCORES          # heads per core
H2 = HPC * H                # 128, both heads' h packed
CS = 512                    # c-chunk (free-dim) width
NCS = SEQ // CS             # 4
NKT = E // 128              # 8 k-tiles over embed
NCT = SEQ // 128            # 16 C-tiles over sequence
SCALE = 1.0 / np.sqrt(H)    # 0.125
F32R = mybir.dt.float32r
F32 = mybir.dt.float32
BF16 = mybir.dt.bfloat16
NPBF16 = ml_dtypes.bfloat16

_built = None
CS_SET = None
NBLK_CAP = 99
DEPTH_OVERRIDE = 9
HOLD_TAIL = 2           # filler units held back for every chunk's normalize
PS_S_BUFS = 2
PS_O_BUFS = 2
HS = CS // 2            # normalize/outproj column-split width (256)
ESB_BUFS = 20           # exp-output tiles in flight
MASK_MOD = 1            # mask TT engine: (d + hh) % MASK_MOD == 0 -> DVE
WARMUP = 0              # PE warmup matmuls during the initial DMA wait
BC_MODE = 0             # bc copies: 0=ACT, 1=DVE, 2=DVE left + ACT right
OSPLIT = False          # outproj copies alternate DVE/ACT per et
TAIL_ET_DMA = True      # per-et DMAs for the final chunk's right outproj
OSPLIT_CS = ()          # chunks whose et=1 outproj copy runs on ACT
MASK_POOL_MAX = 0       # diag blocks with n <= this get their mask on Pool
BC_DVE_CS = ()          # chunks whose bc copies run on DVE
X0_K0 = False           # first chunk-0 xT piece carries only k0
SEL_DRAIN = False       # carry deferred outproj across chunk boundaries


def _build(stage=5):
    nc = bacc.Bacc("TRN2", target_bir_lowering=False, debug=False)

    xT_d = nc.dram_tensor("xT", [E, SEQ], BF16, kind="ExternalInput").ap()
    wq_d = nc.dram_tensor("wq", [128, NKT, H2], BF16, kind="ExternalInput").ap()
    wk_d = nc.dram_tensor("wk", [128, NKT, H2], BF16, kind="ExternalInput").ap()
    wv_d = nc.dram_tensor("wv", [128, NKT, H2], BF16, kind="ExternalInput").ap()
    wo_d = nc.dram_tensor("wo", [H2, E], BF16, kind="ExternalInput").ap()
    masks_d = nc.dram_tensor("masks", [128, CS], BF16, kind="ExternalInput").ap()
    # ones pair (not single column): V1 slabs are padded to 66 bf16
    # elements = 132B so every slab and the ones-pair write are 4-byte
    # aligned -- a lone 2-byte scattered DMA write can RMW-clobber the
    # neighboring tile's first V element while V copies are in flight
    vcol_d = nc.dram_tensor("vcol", [128, HPC, NCT, 2], BF16,
                            kind="ExternalInput").ap()
    ones1_d = nc.dram_tensor("ones1", [1, H], F32R, kind="ExternalInput").ap()
    out_d = nc.dram_tensor("out", [SEQ, E], BF16, kind="ExternalOutput").ap()

    def _body(tc):
        with (
            tc.tile_pool(name="persist", bufs=1) as persist,
            tc.tile_pool(name="work", bufs=3) as work,
            tc.tile_pool(name="zpool", bufs=4) as zpool,
            tc.tile_pool(name="opool", bufs=4) as opool,
            tc.tile_pool(name="ps_proj", bufs=2, space="PSUM") as ps_proj,
            tc.tile_pool(name="ps_s", bufs=PS_S_BUFS, space="PSUM") as ps_s,
            tc.tile_pool(name="ps_z", bufs=2, space="PSUM") as ps_z,
            tc.tile_pool(name="ps_o", bufs=PS_O_BUFS, space="PSUM") as ps_o,
        ):
            # ---- resident tensors -------------------------------------
            xT = persist.tile([128, NKT, SEQ], BF16)
            wq = persist.tile([128, NKT, H2], BF16)
            wk = persist.tile([128, NKT, H2], BF16)
            wv = persist.tile([128, NKT, H2], BF16)
            wo = persist.tile([128, E], BF16)
            ones1 = persist.tile([1, H], F32R)
            QT2 = persist.tile([128, SEQ], BF16)
            KT2 = persist.tile([128, SEQ], BF16)
            V1 = persist.tile([128, HPC, NCT, H + 2], BF16)
            mask_sb = persist.tile([128, CS], BF16)

            # small DMAs first so weights/constants never queue behind the
            # 4MB xT load; xT is issued column-chunk-major so chunk cc's
            # projections start as soon as its columns land.  Each DMA costs
            # 625ns of serialized HWDGE setup, so chunks 1-3 load as a single
            # 3D-AP DMA each; chunk 0 stays per-k so the first projection
            # matmul can start after one small piece
            # DMA issue order tracks first-use time: HWDGE setup is 625ns of
            # serial issue per DMA, so both the issue horizon and the count
            # matter.  Chunk 0 lands in 2-k-tile pieces so the first
            # projection matmul starts early; chunk 1 in two 4-k-tile pieces
            # (its projections are chunk 0's attention filler); chunks 2-3
            # as one DMA each.
            xTv = xT_d.rearrange("(k p) c -> p k c", p=128)
            nc.sync.dma_start(wq[:, 0:NKT // 2, :], wq_d[:, 0:NKT // 2, :])
            if X0_K0:
                nc.sync.dma_start(xT[:, 0:1, 0:CS], xTv[:, 0:1, 0:CS])
                nc.sync.dma_start(xT[:, 1:4, 0:CS], xTv[:, 1:4, 0:CS])
                nc.sync.dma_start(xT[:, 4:8, 0:CS], xTv[:, 4:8, 0:CS])
            else:
                for k2 in range(NKT // 2):
                    nc.sync.dma_start(xT[:, 2 * k2:2 * k2 + 2, 0:CS],
                                      xTv[:, 2 * k2:2 * k2 + 2, 0:CS])
            nc.sync.dma_start(wq[:, NKT // 2:NKT, :], wq_d[:, NKT // 2:NKT, :])
            nc.sync.dma_start(wk[:], wk_d[:])
            nc.sync.dma_start(xT[:, 0:NKT // 2, CS:2 * CS],
                              xTv[:, 0:NKT // 2, CS:2 * CS])
            nc.sync.dma_start(wv[:], wv_d[:])
            nc.sync.dma_start(mask_sb[:], masks_d[:])
            nc.sync.dma_start(xT[:, NKT // 2:NKT, CS:2 * CS],
                              xTv[:, NKT // 2:NKT, CS:2 * CS])
            nc.sync.dma_start(V1[:, :, :, H:H + 2], vcol_d[:])
            nc.sync.dma_start(ones1[:], ones1_d[:])
            nc.sync.dma_start(wo[:], wo_d[:])
            for cc in range(2, NCS):
                nc.sync.dma_start(xT[:, :, cc * CS:(cc + 1) * CS],
                                  xTv[:, :, cc * CS:(cc + 1) * CS])

            def emit_outproj_unit(cs, z2, mt):
                # out[c, e] = z2.T @ wo: K=128 sums both heads at once
                # (z2 rows 0:64 head0, 64:128 head1; wo rows match).
                # Both 512-wide E halves share one 2-bank PSUM tile, one
                # PSUM->SBUF copy and one DMA (HWDGE setup is 625ns each)
                o_sb = opool.tile([128, E], BF16, tag="o", name="o_sb")
                tail = TAIL_ET_DMA and cs == NCS - 1 and mt >= 2
                for et in range(E // CS):
                    o_ps = ps_o.tile([128, CS], F32, name="o_ps")
                    nc.tensor.matmul(
                        o_ps[:],
                        z2[:, mt * 128:(mt + 1) * 128],
                        wo[:, et * CS:(et + 1) * CS],
                        start=True, stop=True,
                    )
                    if et == 1 and (OSPLIT or tail or cs in OSPLIT_CS):
                        nc.scalar.activation(
                            o_sb[:, et * CS:(et + 1) * CS], o_ps[:],
                            mybir.ActivationFunctionType.Copy)
                    else:
                        nc.vector.tensor_copy(o_sb[:, et * CS:(et + 1) * CS],
                                              o_ps[:])
                    if tail:
                        nc.sync.dma_start(
                            out_d[cs * CS + mt * 128:
                                  cs * CS + (mt + 1) * 128,
                                  et * CS:(et + 1) * CS],
                            o_sb[:, et * CS:(et + 1) * CS],
                        )
                if not tail:
                    nc.sync.dma_start(
                        out_d[cs * CS + mt * 128: cs * CS + (mt + 1) * 128, :],
                        o_sb[:],
                    )

            def outproj_units(cs, z2, mts):
                return [(("out", cs), (lambda mt=mt: emit_outproj_unit(cs, z2, mt)))
                        for mt in mts]

            if WARMUP:
                # garbage matmuls on the mask tile: free p-state ramp while
                # the first weight/x pieces are still in flight (results are
                # never read; the scratch PSUM slot is recycled afterwards)
                wu_ps = ps_s.tile([128, CS], F32, tag="s", name="wu_ps")
                for _ in range(WARMUP):
                    nc.tensor.matmul(wu_ps[:], mask_sb[0:128, 0:128],
                                     mask_sb[:, 0:CS], start=True, stop=True)

            def emit_qk_proj(cc, w_sb, dstT, half):
                # one half-projection: accumulate 4 of the 8 embed k-tiles
                c0, c1 = cc * CS, (cc + 1) * CS
                if half == 0:
                    p_ps = ps_proj.tile([128, CS], F32, tag="proj",
                                        name="p_ps")
                    emit_qk_proj.live[(cc, id(dstT))] = p_ps
                else:
                    p_ps = emit_qk_proj.live.pop((cc, id(dstT)))
                for k in range(4 * half, 4 * half + 4):
                    nc.tensor.matmul(
                        p_ps[:], w_sb[:, k, :], xT[:, k, c0:c1],
                        start=(k == 0), stop=(k == NKT - 1),
                    )
                if half == 1:
                    nc.vector.tensor_copy(dstT[:, c0:c1], p_ps[:])
            emit_qk_proj.live = {}

            def emit_v_tile(cc, i):
                # V tile [C, h] directly via lhsT = xT seq-tile: no transpose
                ct = 4 * cc + i
                v_ps = ps_proj.tile([128, HPC, H], F32, tag="proj",
                                    name="v_ps")
                for k in range(NKT):
                    nc.tensor.matmul(
                        v_ps[:], xT[:, k, ct * 128:(ct + 1) * 128],
                        wv[:, k, :],
                        start=(k == 0), stop=(k == NKT - 1),
                    )
                nc.vector.tensor_copy(V1[:, :, ct, 0:H], v_ps[:])

            def proj_units(cc):
                units = [
                    lambda: emit_qk_proj(cc, wq, QT2, 0),
                    lambda: emit_qk_proj(cc, wq, QT2, 1),
                    lambda: emit_qk_proj(cc, wk, KT2, 0),
                    lambda: emit_qk_proj(cc, wk, KT2, 1),
                    lambda: emit_v_tile(cc, 0),
                    lambda: emit_v_tile(cc, 1),
                    lambda: emit_v_tile(cc, 2),
                    lambda: emit_v_tile(cc, 3),
                ]
                return [(("proj", cc), u) for u in units]

            filler = []
            cs_list = list(CS_SET if CS_SET is not None else range(NCS))
            for ci, cc in enumerate(cs_list):
                if ci == 0:
                    for _, f in proj_units(cc):
                        f()

                # ---- attention for cs=cc: both heads' blocks interleaved,
                # diagonal blocks shrunk to their unmasked columns ----------
                cs = cc
                nblk = min(4 * cs + 4, NBLK_CAP)
                z2 = zpool.tile([128, CS], BF16, tag="z", name="z2")
                z_ps = [ps_z.tile([H + 1, CS], F32, tag="zps",
                                  name=f"z_ps{hh}") for hh in range(HPC)]
                # stagger head1 two C-tiles ahead of head0 so the two
                # normalize chains at the end overlap instead of serializing
                OFF = min(2, nblk)
                blocks = []
                for t in range(nblk + OFF):
                    if t < nblk:
                        blocks.append((t, 1))
                    if t >= OFF:
                        blocks.append((t - OFF, 0))
                exp_tiles = {}
                DEPTH = DEPTH_OVERRIDE

                def do_score(i):
                    ct, hh = blocks[i]
                    h0 = hh * H
                    d = ct - 4 * cs
                    off = 128 * d if d > 0 else 0
                    n = CS - off
                    s_ps = ps_s.tile([128, CS], F32, tag="s", name="s_ps")
                    nc.tensor.matmul(
                        s_ps[:, 0:n],
                        QT2[h0:h0 + H, ct * 128:(ct + 1) * 128],
                        KT2[h0:h0 + H, cs * CS + off:(cs + 1) * CS],
                        start=True, stop=True,
                    )
                    e_sb = work.tile([128, CS], BF16, tag="exp",
                                     bufs=ESB_BUFS, name="e_sb")
                    nc.scalar.activation(
                        e_sb[:, 0:n], s_ps[:, 0:n],
                        mybir.ActivationFunctionType.Exp, scale=SCALE,
                    )
                    if d >= 0:
                        # causal: within the shrunk block keep where p <= j.
                        # The diagonal blocks cluster at each chunk's end, so
                        # alternate the SBUF-only multiplies between DVE and
                        # the otherwise-idle Pool to halve the burst rate
                        if n <= MASK_POOL_MAX:
                            eng = nc.gpsimd
                        else:
                            eng = (nc.vector if MASK_MOD == 1
                                   or (d + hh) % MASK_MOD == 0 else nc.gpsimd)
                        eng.tensor_tensor(
                            e_sb[:, 0:n], e_sb[:, 0:n], mask_sb[:, 0:n],
                            op=mybir.AluOpType.mult,
                        )
                    exp_tiles[i] = (e_sb, off, n)

                # normalize z[h, c] /= z[64, c] per 256-wide column half: the
                # left half's PV sum is complete once the d=1 diagonal block
                # lands, so its normalize + output projection overlap the
                # rest of the chunk instead of serializing at the boundary.
                # Only the reciprocal is emitted at the trigger PV; the PE
                # broadcast + copy + multiply become a deferred filler unit
                # so the in-order PE queue never parks on the reciprocal.
                norm_ran = [0, 0]

                def make_norm(hh, side, recip):
                    c0, c1 = side * HS, side * HS + HS

                    def run():
                        b_ps = ps_s.tile([H, HS], F32, tag="s", name="b_ps")
                        nc.tensor.matmul(b_ps[:], ones1[:], recip[:],
                                         start=True, stop=True)
                        bc_sb = work.tile([H, HS], F32R, tag="bc",
                                          name="bc_sb")
                        if cs in BC_DVE_CS:
                            nc.vector.tensor_copy(bc_sb[:], b_ps[:])
                        elif BC_MODE == 0 or (BC_MODE == 2 and side == 1):
                            nc.scalar.activation(
                                bc_sb[:], b_ps[:],
                                mybir.ActivationFunctionType.Copy)
                        else:
                            nc.vector.tensor_copy(bc_sb[:], b_ps[:])
                        nc.vector.tensor_tensor(
                            z2[hh * H:(hh + 1) * H, c0:c1],
                            z_ps[hh][0:H, c0:c1],
                            bc_sb[:], op=mybir.AluOpType.mult,
                        )
                        norm_ran[hh] |= 1 << side
                        if norm_ran[0] & norm_ran[1] & (1 << side):
                            filler.extend(outproj_units(
                                cs, z2, (0, 1) if side == 0 else (2, 3)))
                    return run

                def queue_norm(hh, side):
                    c0, c1 = side * HS, side * HS + HS
                    recip = work.tile([1, HS], F32R, tag="recip",
                                      name="recip")
                    with nc.allow_low_precision("float32r ~ fp32"):
                        nc.vector.reciprocal(recip[:], z_ps[hh][H:H + 1, c0:c1])
                    filler.insert(min(1, len(filler)),
                                  (("norm", cs), make_norm(hh, side, recip)))

                def do_pv(i):
                    ct, hh = blocks[i]
                    e_sb, off, n = exp_tiles.pop(i)
                    nc.tensor.matmul(
                        z_ps[hh][:, off:CS], V1[:, hh, ct, 0:H + 1],
                        e_sb[:, 0:n],
                        start=(ct == 0), stop=(ct == nblk - 1),
                        skip_group_check=True,
                    )
                    if ct == min(4 * cs + 1, nblk - 1):
                        queue_norm(hh, 0)
                    if ct == nblk - 1:
                        queue_norm(hh, 1)

                # drain filler (deferred outproj + NEXT chunk's projections)
                # into this chunk's score/PV pipeline so PE has independent
                # work while ACT computes the exps
                if ci + 1 < len(cs_list):
                    filler.extend(proj_units(cs_list[ci + 1]))
                nb = len(blocks)
                last_cs = ci == len(cs_list) - 1
                # keep a few units in reserve for the chunk's normalize tail,
                # where no score work is left to hide the exp/recip latency
                hold = min(len(filler), DEPTH if last_cs else HOLD_TAIL)
                for i in range(nb):
                    do_score(i)
                    if len(filler) > hold and (i % 2 == 1
                                               or nb - i <= len(filler) - hold):
                        filler.pop(0)[1]()
                    if i >= DEPTH:
                        do_pv(i - DEPTH)
                for i in range(max(0, nb - DEPTH), nb):
                    do_pv(i)
                    if len(filler) > 1 or (filler and last_cs):
                        filler.pop(0)[1]()
                # leftover filler (this chunk's right-half outproj) carries
                # into the next chunk's pipeline -- except projections the
                # next chunk's scores depend on, and this chunk's normalize
                # units (the next chunk's first PV recycles the z_ps pool
                # buffers, which would deadlock the in-order PE queue)
                nxt = cs_list[ci + 1] if ci + 1 < len(cs_list) else None
                if nxt is not None:
                    if SEL_DRAIN:
                        i = 0
                        while i < len(filler):
                            if filler[i][0] in (("proj", nxt), ("norm", cs)):
                                filler.pop(i)[1]()
                            else:
                                i += 1
                    else:
                        while any(tag in (("proj", nxt), ("norm", cs))
                                  for tag, _ in filler):
                            filler.pop(0)[1]()
            while filler:
                filler.pop(0)[1]()

    with tile.TileContext(nc) as tc:
        _body(tc)
    nc.finalize()
    return nc


def _prep_inputs(x, W_Q, W_K, W_V, W_O):
    x = np.asarray(x, dtype=np.float32)
    W_Q = np.asarray(W_Q, dtype=np.float32)
    W_K = np.asarray(W_K, dtype=np.float32)
    W_V = np.asarray(W_V, dtype=np.float32)
    W_O = np.asarray(W_O, dtype=np.float32)

    xT = np.ascontiguousarray(x[0].T).astype(NPBF16)       # [E, SEQ]

    def swz(w):
        # [E, H2] -> [128(p), NKT(k), H2]: p-major so DMA rows are 2KB
        return np.ascontiguousarray(
            w.reshape(NKT, 128, H2).transpose(1, 0, 2)).astype(NPBF16)

    in_maps = []
    for c in range(NCORES):
        a0, a1 = HPC * c, HPC * c + 1
        # [E, 2h]: head0's 64 cols then head1's
        wq = swz(np.concatenate([W_Q[a0].T, W_Q[a1].T], axis=1))
        wk = swz(np.concatenate([W_K[a0].T, W_K[a1].T], axis=1))
        wv = swz(np.concatenate([W_V[a0].T, W_V[a1].T], axis=1))
        # [2h, E]
        wo = np.ascontiguousarray(
            np.concatenate([W_O[a0].T, W_O[a1].T], axis=0)).astype(NPBF16)
        in_maps.append({"xT": xT, "wq": wq, "wk": wk, "wv": wv, "wo": wo,
                        "masks": _MASKS, "vcol": _VCOL, "ones1": _ONES1})
    return in_maps


_MASKS = (np.arange(128)[:, None] <= np.arange(CS)[None, :]).astype(NPBF16)
_VCOL = np.ones((128, HPC, NCT, 2), dtype=NPBF16)
_ONES1 = np.ones((1, H), dtype=np.float32)


def _run(in_maps, trace=False):
    global _built
    if _built is None:
        _built = _build()
    res = bass_utils.run_bass_kernel_spmd(
        _built, in_maps, core_ids=list(range(NCORES)), trace=trace,
    )
    return res


def kernel(x, W_Q, W_K, W_V, W_O):
    in_maps = _prep_inputs(x, W_Q, W_K, W_V, W_O)
    res = _run(in_maps, trace=False)
    acc = np.zeros((SEQ, E), dtype=np.float64)
    for c in range(NCORES):
        acc += np.asarray(res.results[c]["out"], dtype=np.float64)
    return acc.astype(np.float32)[None, :, :]


def kernel_traced(x, W_Q, W_K, W_V, W_O):
    """Like kernel() but also returns a per-core exec-time estimate in ns.

    Prefers a real NTFF profile when the runtime supports it; otherwise
    falls back to the cost-model device-occupancy timeline (TimelineSim),
    since the axon client in this container has no NTFF hook.
    """
    in_maps = _prep_inputs(x, W_Q, W_K, W_V, W_O)
    exec_ns = None
    try:
        res = _run(in_maps, trace=True)
        exec_ns = res.exec_time_ns
    except Exception:
        res = _run(in_maps, trace=False)
    if exec_ns is None:
        from concourse.timeline_sim import TimelineSim
        exec_ns = int(TimelineSim(_built, trace=False).simulate())
    acc = np.zeros((SEQ, E), dtype=np.float64)
    for c in range(NCORES):
        acc += np.asarray(res.results[c]["out"], dtype=np.float64)
    return acc.astype(np.float32)[None, :, :], exec_ns


# revision 43
# speedup vs baseline: 1.1218x; 1.0322x over previous
"""Causal multi-head attention for Trainium2, head-sharded across 8 NeuronCores.

Reference computation (note the role swap: softmax rows are indexed by the
*key* position c and run over *query* positions C <= c):
    q = W_Q x ; k = W_K x ; v = W_V x            per head a
    S[c, C] = k[c] . q[C]
    attn = softmax_C( where(C <= c, S, -inf) / sqrt(64) )
    z[c]  = sum_C attn[c, C] v[C]
    out   = sum_a W_O[a] z[a]                     [seq, embed]

Sharding: 16 heads -> 2 heads per core.  Each core computes the partial
output for its two heads; the host sums the 8 partials.

Device-side design (per core; every matmul's moving operand is bf16 so the
PE streams 1 row/cycle at any width, and DMA traffic is halved):
    xT   [1024, 2048] bf16  x transposed (embed on partitions), streamed in
                            column-chunk-major pieces
    QT2  [128, 2048] bf16   both heads' q-projection, head-major on partitions
    KT2  [128, 2048] bf16   both heads' k-projection
    V1   [128, 2, 16, 65] bf16  v tiles [C, head, ct, h|ones] (col 64 = 1.0)

    V is projected directly in [C, h] orientation (lhsT = xT seq-tile,
    rhs = W_V) so no PE transposes are needed; the ones-column of V1 yields
    the softmax denominator as row 64 of the PV accumulator for free.

    Scores are computed transposed, ST[C, c] = lhsT(QT2).T @ rhs(KT2), so
    the PV contraction (over C) needs no transposes.  exp runs on ACT
    (~0.83 ns/col + ~185 ns/instr), the causal mask multiply and all
    PSUM->SBUF copies on DVE (with normalize broadcast copies on ACT),
    and all DMAs ride the SP HWDGE queue, batched because each costs
    ~625ns of serialized HWDGE setup.

    The whole kernel is one software pipeline over the 4 512-wide c-chunks:
    each chunk's projections run as soon as its xT columns land, its
    attention (both heads' blocks interleaved, diagonal blocks shrunk to
    their unmasked columns) follows immediately, and its output projection
    is deferred into the next chunk's pipeline so no engine drains at the
    chunk boundary.  Output partials are written bf16 and summed on host.
"""

import numpy as np
import ml_dtypes

import concourse.bacc as bacc
import concourse.mybir as mybir
import concourse.tile as tile
from concourse import bass_utils

BATCH, SEQ, E, NH, H = 1, 2048, 1024, 16, 64
NCORES = 8
HPC = NH // NCORES          # heads per core
H2 = HPC * H                # 128, both heads' h packed
CS = 512                    # c-chunk (free-dim) width
NCS = SEQ // CS             # 4
NKT = E // 128              # 8 k-tiles over embed
NCT = SEQ // 128            # 16 C-tiles over sequence
SCALE = 1.0 / np.sqrt(H)    # 0.125
F32R = mybir.dt.float32r
F32 = mybir.dt.float32
BF16 = mybir.dt.bfloat16
NPBF16 = ml_dtypes.bfloat16

_built = None
CS_SET = None
NBLK_CAP = 99
DEPTH_OVERRIDE = 9
HOLD_TAIL = 2           # filler units held back for every chunk's normalize
PS_S_BUFS = 2
PS_O_BUFS = 2
HS = CS // 2            # normalize/outproj column-split width (256)
ESB_BUFS = 20           # exp-output tiles in flight
MASK_MOD = 1            # mask TT engine: (d + hh) % MASK_MOD == 0 -> DVE
WARMUP = 0              # PE warmup matmuls during the initial DMA wait
BC_MODE = 0             # bc copies: 0=ACT, 1=DVE, 2=DVE left + ACT right
OSPLIT = False          # outproj copies alternate DVE/ACT per et
TAIL_ET_DMA = True      # per-et DMAs for the final chunk's right outproj
OSPLIT_CS = ()          # chunks whose et=1 outproj copy runs on ACT
MASK_POOL_MAX = 0       # diag blocks with n <= this get their mask on Pool
BC_DVE_CS = ()          # chunks whose bc copies run on DVE
X0_K0 = False           # first chunk-0 xT piece carries only k0
SEL_DRAIN = True        # carry deferred outproj across chunk boundaries
NORM_POS = 3            # filler insert position for deferred normalize units
DEPTHS = (7, 9, 11, 5)  # per-chunk score->PV pipeline depth
HOLDS = (0, 4, 1, 12)   # per-chunk filler units held for the normalize tail
POPN = (1, 2)           # pop cadence: POPN[0] pops every POPN[1] blocks


def _build(stage=5):
    nc = bacc.Bacc("TRN2", target_bir_lowering=False, debug=False)

    xT_d = nc.dram_tensor("xT", [E, SEQ], BF16, kind="ExternalInput").ap()
    wq_d = nc.dram_tensor("wq", [128, NKT, H2], BF16, kind="ExternalInput").ap()
    wk_d = nc.dram_tensor("wk", [128, NKT, H2], BF16, kind="ExternalInput").ap()
    wv_d = nc.dram_tensor("wv", [128, NKT, H2], BF16, kind="ExternalInput").ap()
    wo_d = nc.dram_tensor("wo", [H2, E], BF16, kind="ExternalInput").ap()
    masks_d = nc.dram_tensor("masks", [128, CS], BF16, kind="ExternalInput").ap()
    # ones pair (not single column): V1 slabs are padded to 66 bf16
    # elements = 132B so every slab and the ones-pair write are 4-byte
    # aligned -- a lone 2-byte scattered DMA write can RMW-clobber the
    # neighboring tile's first V element while V copies are in flight
    vcol_d = nc.dram_tensor("vcol", [128, HPC, NCT, 2], BF16,
                            kind="ExternalInput").ap()
    ones1_d = nc.dram_tensor("ones1", [1, H], F32R, kind="ExternalInput").ap()
    out_d = nc.dram_tensor("out", [SEQ, E], BF16, kind="ExternalOutput").ap()

    def _body(tc):
        with (
            tc.tile_pool(name="persist", bufs=1) as persist,
            tc.tile_pool(name="work", bufs=3) as work,
            tc.tile_pool(name="zpool", bufs=4) as zpool,
            tc.tile_pool(name="opool", bufs=4) as opool,
            tc.tile_pool(name="ps_proj", bufs=2, space="PSUM") as ps_proj,
            tc.tile_pool(name="ps_s", bufs=PS_S_BUFS, space="PSUM") as ps_s,
            tc.tile_pool(name="ps_z", bufs=2, space="PSUM") as ps_z,
            tc.tile_pool(name="ps_o", bufs=PS_O_BUFS, space="PSUM") as ps_o,
        ):
            # ---- resident tensors -------------------------------------
            xT = persist.tile([128, NKT, SEQ], BF16)
            wq = persist.tile([128, NKT, H2], BF16)
            wk = persist.tile([128, NKT, H2], BF16)
            wv = persist.tile([128, NKT, H2], BF16)
            wo = persist.tile([128, E], BF16)
            ones1 = persist.tile([1, H], F32R)
            QT2 = persist.tile([128, SEQ], BF16)
            KT2 = persist.tile([128, SEQ], BF16)
            V1 = persist.tile([128, HPC, NCT, H + 2], BF16)
            mask_sb = persist.tile([128, CS], BF16)

            # DMA issue order tracks first-use time: HWDGE setup is 625ns
            # of serial issue per DMA, so both the issue horizon and the
            # count matter.  Chunk 0 lands in 2-k-tile pieces so the first
            # projection matmul starts early; chunk 1 in two 4-k-tile
            # pieces (its projections are chunk 0's attention filler);
            # chunks 2-3 as one DMA each.
            xTv = xT_d.rearrange("(k p) c -> p k c", p=128)
            nc.sync.dma_start(wq[:, 0:NKT // 2, :], wq_d[:, 0:NKT // 2, :])
            if X0_K0:
                nc.sync.dma_start(xT[:, 0:1, 0:CS], xTv[:, 0:1, 0:CS])
                nc.sync.dma_start(xT[:, 1:4, 0:CS], xTv[:, 1:4, 0:CS])
                nc.sync.dma_start(xT[:, 4:8, 0:CS], xTv[:, 4:8, 0:CS])
            else:
                for k2 in range(NKT // 2):
                    nc.sync.dma_start(xT[:, 2 * k2:2 * k2 + 2, 0:CS],
                                      xTv[:, 2 * k2:2 * k2 + 2, 0:CS])
            nc.sync.dma_start(wq[:, NKT // 2:NKT, :], wq_d[:, NKT // 2:NKT, :])
            nc.sync.dma_start(wk[:], wk_d[:])
            nc.sync.dma_start(xT[:, 0:NKT // 2, CS:2 * CS],
                              xTv[:, 0:NKT // 2, CS:2 * CS])
            nc.sync.dma_start(wv[:], wv_d[:])
            nc.sync.dma_start(mask_sb[:], masks_d[:])
            nc.sync.dma_start(xT[:, NKT // 2:NKT, CS:2 * CS],
                              xTv[:, NKT // 2:NKT, CS:2 * CS])
            nc.sync.dma_start(V1[:, :, :, H:H + 2], vcol_d[:])
            nc.sync.dma_start(ones1[:], ones1_d[:])
            nc.sync.dma_start(wo[:], wo_d[:])
            for cc in range(2, NCS):
                nc.sync.dma_start(xT[:, :, cc * CS:(cc + 1) * CS],
                                  xTv[:, :, cc * CS:(cc + 1) * CS])

            def emit_outproj_unit(cs, z2, mt):
                # out[c, e] = z2.T @ wo: K=128 sums both heads at once
                # (z2 rows 0:64 head0, 64:128 head1; wo rows match).
                # Both 512-wide E halves share one 2-bank PSUM tile, one
                # PSUM->SBUF copy and one DMA (HWDGE setup is 625ns each)
                o_sb = opool.tile([128, E], BF16, tag="o", name="o_sb")
                tail = TAIL_ET_DMA and cs == NCS - 1 and mt >= 2
                for et in range(E // CS):
                    o_ps = ps_o.tile([128, CS], F32, name="o_ps")
                    nc.tensor.matmul(
                        o_ps[:],
                        z2[:, mt * 128:(mt + 1) * 128],
                        wo[:, et * CS:(et + 1) * CS],
                        start=True, stop=True,
                    )
                    if et == 1 and (OSPLIT or tail or cs in OSPLIT_CS):
                        nc.scalar.activation(
                            o_sb[:, et * CS:(et + 1) * CS], o_ps[:],
                            mybir.ActivationFunctionType.Copy)
                    else:
                        nc.vector.tensor_copy(o_sb[:, et * CS:(et + 1) * CS],
                                              o_ps[:])
                    if tail:
                        nc.sync.dma_start(
                            out_d[cs * CS + mt * 128:
                                  cs * CS + (mt + 1) * 128,
                                  et * CS:(et + 1) * CS],
                            o_sb[:, et * CS:(et + 1) * CS],
                        )
                if not tail:
                    nc.sync.dma_start(
                        out_d[cs * CS + mt * 128: cs * CS + (mt + 1) * 128, :],
                        o_sb[:],
                    )

            def outproj_units(cs, z2, mts):
                return [(("out", cs), (lambda mt=mt: emit_outproj_unit(cs, z2, mt)))
                        for mt in mts]

            if WARMUP:
                # garbage matmuls on the mask tile: free p-state ramp while
                # the first weight/x pieces are still in flight (results are
                # never read; the scratch PSUM slot is recycled afterwards)
                wu_ps = ps_s.tile([128, CS], F32, tag="s", name="wu_ps")
                for _ in range(WARMUP):
                    nc.tensor.matmul(wu_ps[:], mask_sb[0:128, 0:128],
                                     mask_sb[:, 0:CS], start=True, stop=True)

            def emit_qk_proj(cc, w_sb, dstT, half):
                # one half-projection: accumulate 4 of the 8 embed k-tiles
                c0, c1 = cc * CS, (cc + 1) * CS
                if half == 0:
                    p_ps = ps_proj.tile([128, CS], F32, tag="proj",
                                        name="p_ps")
                    emit_qk_proj.live[(cc, id(dstT))] = p_ps
                else:
                    p_ps = emit_qk_proj.live.pop((cc, id(dstT)))
                for k in range(4 * half, 4 * half + 4):
                    nc.tensor.matmul(
                        p_ps[:], w_sb[:, k, :], xT[:, k, c0:c1],
                        start=(k == 0), stop=(k == NKT - 1),
                    )
                if half == 1:
                    nc.vector.tensor_copy(dstT[:, c0:c1], p_ps[:])
            emit_qk_proj.live = {}

            def emit_v_tile(cc, i):
                # V tile [C, h] directly via lhsT = xT seq-tile: no transpose
                ct = 4 * cc + i
                v_ps = ps_proj.tile([128, HPC, H], F32, tag="proj",
                                    name="v_ps")
                for k in range(NKT):
                    nc.tensor.matmul(
                        v_ps[:], xT[:, k, ct * 128:(ct + 1) * 128],
                        wv[:, k, :],
                        start=(k == 0), stop=(k == NKT - 1),
                    )
                nc.vector.tensor_copy(V1[:, :, ct, 0:H], v_ps[:])

            def proj_units(cc):
                units = [
                    lambda: emit_qk_proj(cc, wq, QT2, 0),
                    lambda: emit_qk_proj(cc, wq, QT2, 1),
                    lambda: emit_qk_proj(cc, wk, KT2, 0),
                    lambda: emit_qk_proj(cc, wk, KT2, 1),
                    lambda: emit_v_tile(cc, 0),
                    lambda: emit_v_tile(cc, 1),
                    lambda: emit_v_tile(cc, 2),
                    lambda: emit_v_tile(cc, 3),
                ]
                return [(("proj", cc), u) for u in units]

            filler = []
            cs_list = list(CS_SET if CS_SET is not None else range(NCS))
            for ci, cc in enumerate(cs_list):
                if ci == 0:
                    for _, f in proj_units(cc):
                        f()

                # ---- attention for cs=cc: both heads' blocks interleaved,
                # diagonal blocks shrunk to their unmasked columns ----------
                cs = cc
                nblk = min(4 * cs + 4, NBLK_CAP)
                z2 = zpool.tile([128, CS], BF16, tag="z", name="z2")
                z_ps = [ps_z.tile([H + 1, CS], F32, tag="zps",
                                  name=f"z_ps{hh}") for hh in range(HPC)]
                # stagger head1 two C-tiles ahead of head0 so the two
                # normalize chains at the end overlap instead of serializing
                OFF = min(2, nblk)
                blocks = []
                for t in range(nblk + OFF):
                    if t < nblk:
                        blocks.append((t, 1))
                    if t >= OFF:
                        blocks.append((t - OFF, 0))
                exp_tiles = {}
                DEPTH = DEPTHS[ci] if DEPTHS else DEPTH_OVERRIDE

                def do_score(i):
                    ct, hh = blocks[i]
                    h0 = hh * H
                    d = ct - 4 * cs
                    off = 128 * d if d > 0 else 0
                    n = CS - off
                    s_ps = ps_s.tile([128, CS], F32, tag="s", name="s_ps")
                    nc.tensor.matmul(
                        s_ps[:, 0:n],
                        QT2[h0:h0 + H, ct * 128:(ct + 1) * 128],
                        KT2[h0:h0 + H, cs * CS + off:(cs + 1) * CS],
                        start=True, stop=True,
                    )
                    e_sb = work.tile([128, CS], BF16, tag="exp",
                                     bufs=ESB_BUFS, name="e_sb")
                    nc.scalar.activation(
                        e_sb[:, 0:n], s_ps[:, 0:n],
                        mybir.ActivationFunctionType.Exp, scale=SCALE,
                    )
                    if d >= 0:
                        # causal: within the shrunk block keep where p <= j.
                        # The diagonal blocks cluster at each chunk's end, so
                        # alternate the SBUF-only multiplies between DVE and
                        # the otherwise-idle Pool to halve the burst rate
                        if n <= MASK_POOL_MAX:
                            eng = nc.gpsimd
                        else:
                            eng = (nc.vector if MASK_MOD == 1
                                   or (d + hh) % MASK_MOD == 0 else nc.gpsimd)
                        eng.tensor_tensor(
                            e_sb[:, 0:n], e_sb[:, 0:n], mask_sb[:, 0:n],
                            op=mybir.AluOpType.mult,
                        )
                    exp_tiles[i] = (e_sb, off, n)

                # normalize z[h, c] /= z[64, c] per 256-wide column half: the
                # left half's PV sum is complete once the d=1 diagonal block
                # lands, so its normalize + output projection overlap the
                # rest of the chunk instead of serializing at the boundary.
                # Only the reciprocal is emitted at the trigger PV; the PE
                # broadcast + copy + multiply become a deferred filler unit
                # so the in-order PE queue never parks on the reciprocal.
                norm_ran = [0, 0]

                def make_norm(hh, side, recip):
                    c0, c1 = side * HS, side * HS + HS

                    def run():
                        b_ps = ps_s.tile([H, HS], F32, tag="s", name="b_ps")
                        nc.tensor.matmul(b_ps[:], ones1[:], recip[:],
                                         start=True, stop=True)
                        bc_sb = work.tile([H, HS], F32R, tag="bc",
                                          name="bc_sb")
                        if cs in BC_DVE_CS:
                            nc.vector.tensor_copy(bc_sb[:], b_ps[:])
                        elif BC_MODE == 0 or (BC_MODE == 2 and side == 1):
                            nc.scalar.activation(
                                bc_sb[:], b_ps[:],
                                mybir.ActivationFunctionType.Copy)
                        else:
                            nc.vector.tensor_copy(bc_sb[:], b_ps[:])
                        nc.vector.tensor_tensor(
                            z2[hh * H:(hh + 1) * H, c0:c1],
                            z_ps[hh][0:H, c0:c1],
                            bc_sb[:], op=mybir.AluOpType.mult,
                        )
                        norm_ran[hh] |= 1 << side
                        if norm_ran[0] & norm_ran[1] & (1 << side):
                            filler.extend(outproj_units(
                                cs, z2, (0, 1) if side == 0 else (2, 3)))
                    return run

                def queue_norm(hh, side):
                    c0, c1 = side * HS, side * HS + HS
                    recip = work.tile([1, HS], F32R, tag="recip",
                                      name="recip")
                    with nc.allow_low_precision("float32r ~ fp32"):
                        nc.vector.reciprocal(recip[:], z_ps[hh][H:H + 1, c0:c1])
                    filler.insert(min(NORM_POS, len(filler)),
                                  (("norm", cs), make_norm(hh, side, recip)))

                def do_pv(i):
                    ct, hh = blocks[i]
                    e_sb, off, n = exp_tiles.pop(i)
                    nc.tensor.matmul(
                        z_ps[hh][:, off:CS], V1[:, hh, ct, 0:H + 1],
                        e_sb[:, 0:n],
                        start=(ct == 0), stop=(ct == nblk - 1),
                        skip_group_check=True,
                    )
                    if ct == min(4 * cs + 1, nblk - 1):
                        queue_norm(hh, 0)
                    if ct == nblk - 1:
                        queue_norm(hh, 1)

                # drain filler (deferred outproj + NEXT chunk's projections)
                # into this chunk's score/PV pipeline so PE has independent
                # work while ACT computes the exps
                if ci + 1 < len(cs_list):
                    filler.extend(proj_units(cs_list[ci + 1]))
                nb = len(blocks)
                last_cs = ci == len(cs_list) - 1
                # keep a few units in reserve for the chunk's normalize tail,
                # where no score work is left to hide the exp/recip latency
                hold = min(len(filler),
                           (HOLDS[ci] if HOLDS else
                            (DEPTH if last_cs else HOLD_TAIL)))
                for i in range(nb):
                    do_score(i)
                    npop = (POPN[0] if i % POPN[1] == POPN[1] - 1 else 0)
                    if nb - i <= len(filler) - hold:
                        npop = max(npop, 1)
                    for _ in range(npop):
                        if len(filler) > hold:
                            filler.pop(0)[1]()
                    if i >= DEPTH:
                        do_pv(i - DEPTH)
                for i in range(max(0, nb - DEPTH), nb):
                    do_pv(i)
                    if len(filler) > 1 or (filler and last_cs):
                        filler.pop(0)[1]()
                # leftover filler (this chunk's right-half outproj) carries
                # into the next chunk's pipeline -- except projections the
                # next chunk's scores depend on, and this chunk's normalize
                # units (the next chunk's first PV recycles the z_ps pool
                # buffers, which would deadlock the in-order PE queue)
                nxt = cs_list[ci + 1] if ci + 1 < len(cs_list) else None
                if nxt is not None:
                    if SEL_DRAIN:
                        i = 0
                        while i < len(filler):
                            if filler[i][0] in (("proj", nxt), ("norm", cs)):
                                filler.pop(i)[1]()
                            else:
                                i += 1
                    else:
                        while any(tag in (("proj", nxt), ("norm", cs))
                                  for tag, _ in filler):
                            filler.pop(0)[1]()
            while filler:
                filler.pop(0)[1]()

    with tile.TileContext(nc) as tc:
        _body(tc)
    nc.finalize()
    return nc


def _prep_inputs(x, W_Q, W_K, W_V, W_O):
    x = np.asarray(x, dtype=np.float32)
    W_Q = np.asarray(W_Q, dtype=np.float32)
    W_K = np.asarray(W_K, dtype=np.float32)
    W_V = np.asarray(W_V, dtype=np.float32)
    W_O = np.asarray(W_O, dtype=np.float32)

    xT = np.ascontiguousarray(x[0].T).astype(NPBF16)       # [E, SEQ]

    def swz(w):
        # [E, H2] -> [128(p), NKT(k), H2]: p-major so DMA rows are 2KB
        return np.ascontiguousarray(
            w.reshape(NKT, 128, H2).transpose(1, 0, 2)).astype(NPBF16)

    in_maps = []
    for c in range(NCORES):
        a0, a1 = HPC * c, HPC * c + 1
        # [E, 2h]: head0's 64 cols then head1's
        wq = swz(np.concatenate([W_Q[a0].T, W_Q[a1].T], axis=1))
        wk = swz(np.concatenate([W_K[a0].T, W_K[a1].T], axis=1))
        wv = swz(np.concatenate([W_V[a0].T, W_V[a1].T], axis=1))
        # [2h, E]
        wo = np.ascontiguousarray(
            np.concatenate([W_O[a0].T, W_O[a1].T], axis=0)).astype(NPBF16)
        in_maps.append({"xT": xT, "wq": wq, "wk": wk, "wv": wv, "wo": wo,
                        "masks": _MASKS, "vcol": _VCOL, "ones1": _ONES1})
    return in_maps


_MASKS = (np.arange(128)[:, None] <= np.arange(CS)[None, :]).astype(NPBF16)
_VCOL = np.ones((128, HPC, NCT, 2), dtype=NPBF16)
_ONES1 = np.ones((1, H), dtype=np.float32)


def _run(in_maps, trace=False):
    global _built
    if _built is None:
        _built = _build()
    res = bass_utils.run_bass_kernel_spmd(
        _built, in_maps, core_ids=list(range(NCORES)), trace=trace,
    )
    return res


def kernel(x, W_Q, W_K, W_V, W_O):
    in_maps = _prep_inputs(x, W_Q, W_K, W_V, W_O)
    res = _run(in_maps, trace=False)
    acc = np.zeros((SEQ, E), dtype=np.float64)
    for c in range(NCORES):
        acc += np.asarray(res.results[c]["out"], dtype=np.float64)
    return acc.astype(np.float32)[None, :, :]


def kernel_traced(x, W_Q, W_K, W_V, W_O):
    """Like kernel() but also returns a per-core exec-time estimate in ns.

    Prefers a real NTFF profile when the runtime supports it; otherwise
    falls back to the cost-model device-occupancy timeline (TimelineSim),
    since the axon client in this container has no NTFF hook.
    """
    in_maps = _prep_inputs(x, W_Q, W_K, W_V, W_O)
    exec_ns = None
    try:
        res = _run(in_maps, trace=True)
        exec_ns = res.exec_time_ns
    except Exception:
        res = _run(in_maps, trace=False)
    if exec_ns is None:
        from concourse.timeline_sim import TimelineSim
        exec_ns = int(TimelineSim(_built, trace=False).simulate())
    acc = np.zeros((SEQ, E), dtype=np.float64)
    for c in range(NCORES):
        acc += np.asarray(res.results[c]["out"], dtype=np.float64)
    return acc.astype(np.float32)[None, :, :], exec_ns


# revision 46
# speedup vs baseline: 1.1305x; 1.0078x over previous
"""Causal multi-head attention for Trainium2, head-sharded across 8 NeuronCores.

Reference computation (note the role swap: softmax rows are indexed by the
*key* position c and run over *query* positions C <= c):
    q = W_Q x ; k = W_K x ; v = W_V x            per head a
    S[c, C] = k[c] . q[C]
    attn = softmax_C( where(C <= c, S, -inf) / sqrt(64) )
    z[c]  = sum_C attn[c, C] v[C]
    out   = sum_a W_O[a] z[a]                     [seq, embed]

Sharding: 16 heads -> 2 heads per core.  Each core computes the partial
output for its two heads; the host sums the 8 partials.

Device-side design (per core; every matmul's moving operand is bf16 so the
PE streams 1 row/cycle at any width, and DMA traffic is halved):
    xT   [1024, 2048] bf16  x transposed (embed on partitions), streamed in
                            column-chunk-major pieces
    QT2  [128, 2048] bf16   both heads' q-projection, head-major on partitions
    KT2  [128, 2048] bf16   both heads' k-projection
    V1   [128, 2, 16, 65] bf16  v tiles [C, head, ct, h|ones] (col 64 = 1.0)

    V is projected directly in [C, h] orientation (lhsT = xT seq-tile,
    rhs = W_V) so no PE transposes are needed; the ones-column of V1 yields
    the softmax denominator as row 64 of the PV accumulator for free.

    Scores are computed transposed, ST[C, c] = lhsT(QT2).T @ rhs(KT2), so
    the PV contraction (over C) needs no transposes.  exp runs on ACT
    (~0.83 ns/col + ~185 ns/instr), the causal mask multiply and all
    PSUM->SBUF copies on DVE (with normalize broadcast copies on ACT),
    and all DMAs ride the SP HWDGE queue, batched because each costs
    ~625ns of serialized HWDGE setup.

    The whole kernel is one software pipeline over the 4 512-wide c-chunks:
    each chunk's projections run as soon as its xT columns land, its
    attention (both heads' blocks interleaved, diagonal blocks shrunk to
    their unmasked columns) follows immediately, and its output projection
    is deferred into the next chunk's pipeline so no engine drains at the
    chunk boundary.  Output partials are written bf16 and summed on host.
"""

import numpy as np
import ml_dtypes

import concourse.bacc as bacc
import concourse.mybir as mybir
import concourse.tile as tile
from concourse import bass_utils

BATCH, SEQ, E, NH, H = 1, 2048, 1024, 16, 64
NCORES = 8
HPC = NH // NCORES          # heads per core
H2 = HPC * H                # 128, both heads' h packed
CS = 512                    # c-chunk (free-dim) width
NCS = SEQ // CS             # 4
NKT = E // 128              # 8 k-tiles over embed
NCT = SEQ // 128            # 16 C-tiles over sequence
SCALE = 1.0 / np.sqrt(H)    # 0.125
F32R = mybir.dt.float32r
F32 = mybir.dt.float32
BF16 = mybir.dt.bfloat16
NPBF16 = ml_dtypes.bfloat16

_built = None
CS_SET = None
NBLK_CAP = 99
DEPTH_OVERRIDE = 9
HOLD_TAIL = 2           # filler units held back for every chunk's normalize
PS_S_BUFS = 2
PS_O_BUFS = 2
HS = CS // 2            # normalize/outproj column-split width (256)
ESB_BUFS = 20           # exp-output tiles in flight
MASK_MOD = 1            # mask TT engine: (d + hh) % MASK_MOD == 0 -> DVE
WARMUP = 0              # PE warmup matmuls during the initial DMA wait
BC_MODE = 0             # bc copies: 0=ACT, 1=DVE, 2=DVE left + ACT right
OSPLIT = False          # outproj copies alternate DVE/ACT per et
TAIL_ET_DMA = True      # per-et DMAs for the final chunk's right outproj
OSPLIT_CS = ()          # chunks whose et=1 outproj copy runs on ACT
MASK_POOL_MAX = 0       # diag blocks with n <= this get their mask on Pool
BC_DVE_CS = ()          # chunks whose bc copies run on DVE
X0_K0 = False           # first chunk-0 xT piece carries only k0
SEL_DRAIN = True        # carry deferred outproj across chunk boundaries
NORM_POS = 3            # filler insert position for deferred normalize units
DEPTHS = (5, 9, 11, 5)  # per-chunk score->PV pipeline depth
HOLDS = (0, 4, 1, 12)   # per-chunk filler units held for the normalize tail
POPN = (1, 2)           # pop cadence: POPN[0] pops every POPN[1] blocks
HEAD_K0 = False         # first wq/xT pieces carry only k0
TAIL3 = False           # last chunk: 3-way normalize split
TAIL_PSPROJ = True      # tail outproj et=1 borrows the idle ps_proj slot


def _build(stage=5):
    nc = bacc.Bacc("TRN2", target_bir_lowering=False, debug=False)

    xT_d = nc.dram_tensor("xT", [E, SEQ], BF16, kind="ExternalInput").ap()
    wq_d = nc.dram_tensor("wq", [128, NKT, H2], BF16, kind="ExternalInput").ap()
    wk_d = nc.dram_tensor("wk", [128, NKT, H2], BF16, kind="ExternalInput").ap()
    wv_d = nc.dram_tensor("wv", [128, NKT, H2], BF16, kind="ExternalInput").ap()
    wo_d = nc.dram_tensor("wo", [H2, E], BF16, kind="ExternalInput").ap()
    masks_d = nc.dram_tensor("masks", [128, CS], BF16, kind="ExternalInput").ap()
    # ones pair (not single column): V1 slabs are padded to 66 bf16
    # elements = 132B so every slab and the ones-pair write are 4-byte
    # aligned -- a lone 2-byte scattered DMA write can RMW-clobber the
    # neighboring tile's first V element while V copies are in flight
    vcol_d = nc.dram_tensor("vcol", [128, HPC, NCT, 2], BF16,
                            kind="ExternalInput").ap()
    ones1_d = nc.dram_tensor("ones1", [1, H], F32R, kind="ExternalInput").ap()
    out_d = nc.dram_tensor("out", [SEQ, E], BF16, kind="ExternalOutput").ap()

    def _body(tc):
        with (
            tc.tile_pool(name="persist", bufs=1) as persist,
            tc.tile_pool(name="work", bufs=3) as work,
            tc.tile_pool(name="zpool", bufs=4) as zpool,
            tc.tile_pool(name="opool", bufs=4) as opool,
            tc.tile_pool(name="ps_proj", bufs=2, space="PSUM") as ps_proj,
            tc.tile_pool(name="ps_s", bufs=PS_S_BUFS, space="PSUM") as ps_s,
            tc.tile_pool(name="ps_z", bufs=2, space="PSUM") as ps_z,
            tc.tile_pool(name="ps_o", bufs=PS_O_BUFS, space="PSUM") as ps_o,
        ):
            # ---- resident tensors -------------------------------------
            xT = persist.tile([128, NKT, SEQ], BF16)
            wq = persist.tile([128, NKT, H2], BF16)
            wk = persist.tile([128, NKT, H2], BF16)
            wv = persist.tile([128, NKT, H2], BF16)
            wo = persist.tile([128, E], BF16)
            ones1 = persist.tile([1, H], F32R)
            QT2 = persist.tile([128, SEQ], BF16)
            KT2 = persist.tile([128, SEQ], BF16)
            V1 = persist.tile([128, HPC, NCT, H + 2], BF16)
            mask_sb = persist.tile([128, CS], BF16)

            # DMA issue order tracks first-use time: HWDGE setup is 625ns
            # of serial issue per DMA, so both the issue horizon and the
            # count matter.  Chunk 0 lands in 2-k-tile pieces so the first
            # projection matmul starts early; chunk 1 in two 4-k-tile
            # pieces (its projections are chunk 0's attention filler);
            # chunks 2-3 as one DMA each.
            xTv = xT_d.rearrange("(k p) c -> p k c", p=128)
            if HEAD_K0:
                nc.sync.dma_start(wq[:, 0:1, :], wq_d[:, 0:1, :])
                nc.sync.dma_start(xT[:, 0:1, 0:CS], xTv[:, 0:1, 0:CS])
                nc.sync.dma_start(wq[:, 1:NKT // 2, :], wq_d[:, 1:NKT // 2, :])
                nc.sync.dma_start(xT[:, 1:4, 0:CS], xTv[:, 1:4, 0:CS])
                nc.sync.dma_start(xT[:, 4:8, 0:CS], xTv[:, 4:8, 0:CS])
            else:
                nc.sync.dma_start(wq[:, 0:NKT // 2, :], wq_d[:, 0:NKT // 2, :])
            if HEAD_K0:
                pass
            elif X0_K0:
                nc.sync.dma_start(xT[:, 0:1, 0:CS], xTv[:, 0:1, 0:CS])
                nc.sync.dma_start(xT[:, 1:4, 0:CS], xTv[:, 1:4, 0:CS])
                nc.sync.dma_start(xT[:, 4:8, 0:CS], xTv[:, 4:8, 0:CS])
            else:
                for k2 in range(NKT // 2):
                    nc.sync.dma_start(xT[:, 2 * k2:2 * k2 + 2, 0:CS],
                                      xTv[:, 2 * k2:2 * k2 + 2, 0:CS])
            nc.sync.dma_start(wq[:, NKT // 2:NKT, :], wq_d[:, NKT // 2:NKT, :])
            nc.sync.dma_start(wk[:], wk_d[:])
            nc.sync.dma_start(xT[:, 0:NKT // 2, CS:2 * CS],
                              xTv[:, 0:NKT // 2, CS:2 * CS])
            nc.sync.dma_start(wv[:], wv_d[:])
            nc.sync.dma_start(mask_sb[:], masks_d[:])
            nc.sync.dma_start(xT[:, NKT // 2:NKT, CS:2 * CS],
                              xTv[:, NKT // 2:NKT, CS:2 * CS])
            nc.sync.dma_start(V1[:, :, :, H:H + 2], vcol_d[:])
            nc.sync.dma_start(ones1[:], ones1_d[:])
            nc.sync.dma_start(wo[:], wo_d[:])
            for cc in range(2, NCS):
                nc.sync.dma_start(xT[:, :, cc * CS:(cc + 1) * CS],
                                  xTv[:, :, cc * CS:(cc + 1) * CS])

            def emit_outproj_unit(cs, z2, mt):
                # out[c, e] = z2.T @ wo: K=128 sums both heads at once
                # (z2 rows 0:64 head0, 64:128 head1; wo rows match).
                # Both 512-wide E halves share one 2-bank PSUM tile, one
                # PSUM->SBUF copy and one DMA (HWDGE setup is 625ns each)
                o_sb = opool.tile([128, E], BF16, tag="o", name="o_sb")
                tail = TAIL_ET_DMA and cs == NCS - 1 and mt >= 2
                for et in range(E // CS):
                    if TAIL_PSPROJ and tail and et == 1:
                        o_ps = ps_proj.tile([128, CS], F32, tag="proj",
                                            name="o_ps")
                    else:
                        o_ps = ps_o.tile([128, CS], F32, name="o_ps")
                    nc.tensor.matmul(
                        o_ps[:],
                        z2[:, mt * 128:(mt + 1) * 128],
                        wo[:, et * CS:(et + 1) * CS],
                        start=True, stop=True,
                    )
                    if et == 1 and (OSPLIT or tail or cs in OSPLIT_CS):
                        nc.scalar.activation(
                            o_sb[:, et * CS:(et + 1) * CS], o_ps[:],
                            mybir.ActivationFunctionType.Copy)
                    else:
                        nc.vector.tensor_copy(o_sb[:, et * CS:(et + 1) * CS],
                                              o_ps[:])
                    if tail:
                        nc.sync.dma_start(
                            out_d[cs * CS + mt * 128:
                                  cs * CS + (mt + 1) * 128,
                                  et * CS:(et + 1) * CS],
                            o_sb[:, et * CS:(et + 1) * CS],
                        )
                if not tail:
                    nc.sync.dma_start(
                        out_d[cs * CS + mt * 128: cs * CS + (mt + 1) * 128, :],
                        o_sb[:],
                    )

            def outproj_units(cs, z2, mts):
                return [(("out", cs), (lambda mt=mt: emit_outproj_unit(cs, z2, mt)))
                        for mt in mts]

            if WARMUP:
                # garbage matmuls on the mask tile: free p-state ramp while
                # the first weight/x pieces are still in flight (results are
                # never read; the scratch PSUM slot is recycled afterwards)
                wu_ps = ps_s.tile([128, CS], F32, tag="s", name="wu_ps")
                for _ in range(WARMUP):
                    nc.tensor.matmul(wu_ps[:], mask_sb[0:128, 0:128],
                                     mask_sb[:, 0:CS], start=True, stop=True)

            def emit_qk_proj(cc, w_sb, dstT, half):
                # one half-projection: accumulate 4 of the 8 embed k-tiles
                c0, c1 = cc * CS, (cc + 1) * CS
                if half == 0:
                    p_ps = ps_proj.tile([128, CS], F32, tag="proj",
                                        name="p_ps")
                    emit_qk_proj.live[(cc, id(dstT))] = p_ps
                else:
                    p_ps = emit_qk_proj.live.pop((cc, id(dstT)))
                for k in range(4 * half, 4 * half + 4):
                    nc.tensor.matmul(
                        p_ps[:], w_sb[:, k, :], xT[:, k, c0:c1],
                        start=(k == 0), stop=(k == NKT - 1),
                    )
                if half == 1:
                    nc.vector.tensor_copy(dstT[:, c0:c1], p_ps[:])
            emit_qk_proj.live = {}

            def emit_v_tile(cc, i):
                # V tile [C, h] directly via lhsT = xT seq-tile: no transpose
                ct = 4 * cc + i
                v_ps = ps_proj.tile([128, HPC, H], F32, tag="proj",
                                    name="v_ps")
                for k in range(NKT):
                    nc.tensor.matmul(
                        v_ps[:], xT[:, k, ct * 128:(ct + 1) * 128],
                        wv[:, k, :],
                        start=(k == 0), stop=(k == NKT - 1),
                    )
                nc.vector.tensor_copy(V1[:, :, ct, 0:H], v_ps[:])

            def proj_units(cc):
                units = [
                    lambda: emit_qk_proj(cc, wq, QT2, 0),
                    lambda: emit_qk_proj(cc, wq, QT2, 1),
                    lambda: emit_qk_proj(cc, wk, KT2, 0),
                    lambda: emit_qk_proj(cc, wk, KT2, 1),
                    lambda: emit_v_tile(cc, 0),
                    lambda: emit_v_tile(cc, 1),
                    lambda: emit_v_tile(cc, 2),
                    lambda: emit_v_tile(cc, 3),
                ]
                return [(("proj", cc), u) for u in units]

            filler = []
            cs_list = list(CS_SET if CS_SET is not None else range(NCS))
            for ci, cc in enumerate(cs_list):
                if ci == 0:
                    for _, f in proj_units(cc):
                        f()

                # ---- attention for cs=cc: both heads' blocks interleaved,
                # diagonal blocks shrunk to their unmasked columns ----------
                cs = cc
                nblk = min(4 * cs + 4, NBLK_CAP)
                z2 = zpool.tile([128, CS], BF16, tag="z", name="z2")
                z_ps = [ps_z.tile([H + 1, CS], F32, tag="zps",
                                  name=f"z_ps{hh}") for hh in range(HPC)]
                # stagger head1 two C-tiles ahead of head0 so the two
                # normalize chains at the end overlap instead of serializing
                OFF = min(2, nblk)
                blocks = []
                for t in range(nblk + OFF):
                    if t < nblk:
                        blocks.append((t, 1))
                    if t >= OFF:
                        blocks.append((t - OFF, 0))
                exp_tiles = {}
                DEPTH = DEPTHS[ci] if DEPTHS else DEPTH_OVERRIDE

                def do_score(i):
                    ct, hh = blocks[i]
                    h0 = hh * H
                    d = ct - 4 * cs
                    off = 128 * d if d > 0 else 0
                    n = CS - off
                    s_ps = ps_s.tile([128, CS], F32, tag="s", name="s_ps")
                    nc.tensor.matmul(
                        s_ps[:, 0:n],
                        QT2[h0:h0 + H, ct * 128:(ct + 1) * 128],
                        KT2[h0:h0 + H, cs * CS + off:(cs + 1) * CS],
                        start=True, stop=True,
                    )
                    e_sb = work.tile([128, CS], BF16, tag="exp",
                                     bufs=ESB_BUFS, name="e_sb")
                    nc.scalar.activation(
                        e_sb[:, 0:n], s_ps[:, 0:n],
                        mybir.ActivationFunctionType.Exp, scale=SCALE,
                    )
                    if d >= 0:
                        # causal: within the shrunk block keep where p <= j.
                        # The diagonal blocks cluster at each chunk's end, so
                        # alternate the SBUF-only multiplies between DVE and
                        # the otherwise-idle Pool to halve the burst rate
                        if n <= MASK_POOL_MAX:
                            eng = nc.gpsimd
                        else:
                            eng = (nc.vector if MASK_MOD == 1
                                   or (d + hh) % MASK_MOD == 0 else nc.gpsimd)
                        eng.tensor_tensor(
                            e_sb[:, 0:n], e_sb[:, 0:n], mask_sb[:, 0:n],
                            op=mybir.AluOpType.mult,
                        )
                    exp_tiles[i] = (e_sb, off, n)

                # normalize z[h, c] /= z[64, c] per 256-wide column half: the
                # left half's PV sum is complete once the d=1 diagonal block
                # lands, so its normalize + output projection overlap the
                # rest of the chunk instead of serializing at the boundary.
                # Only the reciprocal is emitted at the trigger PV; the PE
                # broadcast + copy + multiply become a deferred filler unit
                # so the in-order PE queue never parks on the reciprocal.
                norm_ran = [0, 0]

                def nrange(side):
                    if TAIL3 and ci == len(cs_list) - 1:
                        return ((0, 2 * 128, (0, 1)), (2 * 128, 3 * 128, (2,)),
                                (3 * 128, 4 * 128, (3,)))[side]
                    return ((0, HS, (0, 1)), (HS, CS, (2, 3)))[side]

                def make_norm(hh, side, recip):
                    c0, c1, mts = nrange(side)

                    def run():
                        b_ps = ps_s.tile([H, HS], F32, tag="s", name="b_ps")
                        nc.tensor.matmul(b_ps[:, 0:c1 - c0], ones1[:],
                                         recip[:, 0:c1 - c0],
                                         start=True, stop=True)
                        bc_sb = work.tile([H, HS], F32R, tag="bc",
                                          name="bc_sb")
                        if cs in BC_DVE_CS:
                            nc.vector.tensor_copy(bc_sb[:, 0:c1 - c0],
                                                  b_ps[:, 0:c1 - c0])
                        elif BC_MODE == 0 or (BC_MODE == 2 and side == 1):
                            nc.scalar.activation(
                                bc_sb[:, 0:c1 - c0], b_ps[:, 0:c1 - c0],
                                mybir.ActivationFunctionType.Copy)
                        else:
                            nc.vector.tensor_copy(bc_sb[:, 0:c1 - c0],
                                                  b_ps[:, 0:c1 - c0])
                        nc.vector.tensor_tensor(
                            z2[hh * H:(hh + 1) * H, c0:c1],
                            z_ps[hh][0:H, c0:c1],
                            bc_sb[:, 0:c1 - c0], op=mybir.AluOpType.mult,
                        )
                        norm_ran[hh] |= 1 << side
                        if norm_ran[0] & norm_ran[1] & (1 << side):
                            filler.extend(outproj_units(cs, z2, mts))
                    return run

                def queue_norm(hh, side):
                    c0, c1, _ = nrange(side)
                    recip = work.tile([1, HS], F32R, tag="recip",
                                      name="recip")
                    with nc.allow_low_precision("float32r ~ fp32"):
                        nc.vector.reciprocal(recip[:, 0:c1 - c0],
                                             z_ps[hh][H:H + 1, c0:c1])
                    filler.insert(min(NORM_POS, len(filler)),
                                  (("norm", cs), make_norm(hh, side, recip)))

                def do_pv(i):
                    ct, hh = blocks[i]
                    e_sb, off, n = exp_tiles.pop(i)
                    nc.tensor.matmul(
                        z_ps[hh][:, off:CS], V1[:, hh, ct, 0:H + 1],
                        e_sb[:, 0:n],
                        start=(ct == 0), stop=(ct == nblk - 1),
                        skip_group_check=True,
                    )
                    nsides = 3 if (TAIL3 and last_cs) else 2
                    for side in range(nsides):
                        if nsides == 3:
                            t = min(4 * cs + 1 + side, nblk - 1)
                            prev = min(4 * cs + side, nblk - 1) if side else -1
                        else:
                            t = (min(4 * cs + 1, nblk - 1) if side == 0
                                 else nblk - 1)
                            prev = min(4 * cs + 1, nblk - 1) if side else -1
                        if ct == t and t > prev:
                            queue_norm(hh, side)

                # drain filler (deferred outproj + NEXT chunk's projections)
                # into this chunk's score/PV pipeline so PE has independent
                # work while ACT computes the exps
                if ci + 1 < len(cs_list):
                    filler.extend(proj_units(cs_list[ci + 1]))
                nb = len(blocks)
                last_cs = ci == len(cs_list) - 1
                # keep a few units in reserve for the chunk's normalize tail,
                # where no score work is left to hide the exp/recip latency
                hold = min(len(filler),
                           (HOLDS[ci] if HOLDS else
                            (DEPTH if last_cs else HOLD_TAIL)))
                for i in range(nb):
                    do_score(i)
                    npop = (POPN[0] if i % POPN[1] == POPN[1] - 1 else 0)
                    if nb - i <= len(filler) - hold:
                        npop = max(npop, 1)
                    for _ in range(npop):
                        if len(filler) > hold:
                            filler.pop(0)[1]()
                    if i >= DEPTH:
                        do_pv(i - DEPTH)
                for i in range(max(0, nb - DEPTH), nb):
                    do_pv(i)
                    if len(filler) > 1 or (filler and last_cs):
                        filler.pop(0)[1]()
                # leftover filler (this chunk's right-half outproj) carries
                # into the next chunk's pipeline -- except projections the
                # next chunk's scores depend on, and this chunk's normalize
                # units (the next chunk's first PV recycles the z_ps pool
                # buffers, which would deadlock the in-order PE queue)
                nxt = cs_list[ci + 1] if ci + 1 < len(cs_list) else None
                if nxt is not None:
                    if SEL_DRAIN:
                        i = 0
                        while i < len(filler):
                            if filler[i][0] in (("proj", nxt), ("norm", cs)):
                                filler.pop(i)[1]()
                            else:
                                i += 1
                    else:
                        while any(tag in (("proj", nxt), ("norm", cs))
                                  for tag, _ in filler):
                            filler.pop(0)[1]()
            while filler:
                filler.pop(0)[1]()

    with tile.TileContext(nc) as tc:
        _body(tc)
    nc.finalize()
    return nc


def _prep_inputs(x, W_Q, W_K, W_V, W_O):
    x = np.asarray(x, dtype=np.float32)
    W_Q = np.asarray(W_Q, dtype=np.float32)
    W_K = np.asarray(W_K, dtype=np.float32)
    W_V = np.asarray(W_V, dtype=np.float32)
    W_O = np.asarray(W_O, dtype=np.float32)

    xT = np.ascontiguousarray(x[0].T).astype(NPBF16)       # [E, SEQ]

    def swz(w):
        # [E, H2] -> [128(p), NKT(k), H2]: p-major so DMA rows are 2KB
        return np.ascontiguousarray(
            w.reshape(NKT, 128, H2).transpose(1, 0, 2)).astype(NPBF16)

    in_maps = []
    for c in range(NCORES):
        a0, a1 = HPC * c, HPC * c + 1
        # [E, 2h]: head0's 64 cols then head1's
        wq = swz(np.concatenate([W_Q[a0].T, W_Q[a1].T], axis=1))
        wk = swz(np.concatenate([W_K[a0].T, W_K[a1].T], axis=1))
        wv = swz(np.concatenate([W_V[a0].T, W_V[a1].T], axis=1))
        # [2h, E]
        wo = np.ascontiguousarray(
            np.concatenate([W_O[a0].T, W_O[a1].T], axis=0)).astype(NPBF16)
        in_maps.append({"xT": xT, "wq": wq, "wk": wk, "wv": wv, "wo": wo,
                        "masks": _MASKS, "vcol": _VCOL, "ones1": _ONES1})
    return in_maps


_MASKS = (np.arange(128)[:, None] <= np.arange(CS)[None, :]).astype(NPBF16)
_VCOL = np.ones((128, HPC, NCT, 2), dtype=NPBF16)
_ONES1 = np.ones((1, H), dtype=np.float32)


def _run(in_maps, trace=False):
    global _built
    if _built is None:
        _built = _build()
    res = bass_utils.run_bass_kernel_spmd(
        _built, in_maps, core_ids=list(range(NCORES)), trace=trace,
    )
    return res


def kernel(x, W_Q, W_K, W_V, W_O):
    in_maps = _prep_inputs(x, W_Q, W_K, W_V, W_O)
    res = _run(in_maps, trace=False)
    acc = np.zeros((SEQ, E), dtype=np.float64)
    for c in range(NCORES):
        acc += np.asarray(res.results[c]["out"], dtype=np.float64)
    return acc.astype(np.float32)[None, :, :]


def kernel_traced(x, W_Q, W_K, W_V, W_O):
    """Like kernel() but also returns a per-core exec-time estimate in ns.

    Prefers a real NTFF profile when the runtime supports it; otherwise
    falls back to the cost-model device-occupancy timeline (TimelineSim),
    since the axon client in this container has no NTFF hook.
    """
    in_maps = _prep_inputs(x, W_Q, W_K, W_V, W_O)
    exec_ns = None
    try:
        res = _run(in_maps, trace=True)
        exec_ns = res.exec_time_ns
    except Exception:
        res = _run(in_maps, trace=False)
    if exec_ns is None:
        from concourse.timeline_sim import TimelineSim
        exec_ns = int(TimelineSim(_built, trace=False).simulate())
    acc = np.zeros((SEQ, E), dtype=np.float64)
    for c in range(NCORES):
        acc += np.asarray(res.results[c]["out"], dtype=np.float64)
    return acc.astype(np.float32)[None, :, :], exec_ns
